# revision 35
# baseline (speedup 1.0000x reference)
"""DeltaNet attention TRN2 kernel (nn_DeltaNetAttention_5299989643476).

Strategy: data-parallel over batch (8 batches -> 8 NeuronCores). The
cross-batch cumulative_state scan is tiny ([H, Dh]) and is computed on the
host via an algebraic shortcut (mean over (b,l) of kv == Ksum . V
contraction), then passed to every core as a small constant tensor, so the
device program needs no collectives.

On-device, everything runs in a "transposed" layout (features on
partitions, sequence on the free dim):
  - QT/KT/VT projections: weight-stationary fp8e4 DoubleRow matmuls (2x PE
    throughput), fp32 PSUM accum; weights host-prescaled by SW=64 to clear
    the fp8 denormal range, unscaled in the PSUM->SBUF copy
  - per head: kvT matmul; q-mod via tensor_scalar with per-partition cs;
    phi(x)=elu(x)+1 = relu(x)+exp(min(x,0)); causal linear attention as a
    masked A=pq@pk^T matmul; den via an all-ones stationary matmul (which
    also replicates den across partitions for the division broadcast);
    num needs V back in sequence-major layout -> PE transpose
  - output projection + residual + LayerNorm (bn_stats/bn_aggr)
"""

import numpy as np
import ml_dtypes

import concourse.bass as bass
import concourse.mybir as mybir
import concourse.tile as tile
from concourse import bacc
from concourse.bass_utils import run_bass_kernel_spmd
from concourse.masks import make_identity


def _ensure_axon_hooks():
    """This image's `antenv` lacks `axon_hooks`; if the caller's environment
    sets BASS_TRACE, run_bass_kernel_spmd would crash importing it. Register
    a no-op shim (only when absent) so tracing degrades gracefully."""
    try:
        import antenv.axon_hooks  # noqa: F401
    except ImportError:
        import sys
        import types

        import antenv

        mod = types.ModuleType("antenv.axon_hooks")
        _h = [None]
        mod.set_axon_ntff_profile_hook = lambda h: _h.__setitem__(0, h)
        mod.get_axon_ntff_profile_hook = lambda: _h[0]
        sys.modules["antenv.axon_hooks"] = mod
        antenv.axon_hooks = mod


_ensure_axon_hooks()

B, L, D, H = 8, 256, 2048, 8
DH = D // H            # 256
NB = D // 128          # 16 feature blocks of 128
LB = L // 128          # 2 sequence blocks of 128
EPS = 1e-5
SW = 64.0              # fp8 weight pre-scale (power of 2: exact)
SA = 32.0              # fp8 attn pre-scale; SW*SA is folded into qres

F32 = mybir.dt.float32
F16 = mybir.dt.float16
BF16 = mybir.dt.bfloat16
F8 = mybir.dt.float8e4
AF = mybir.ActivationFunctionType
OP = mybir.AluOpType
DR = mybir.MatmulPerfMode.DoubleRow

_cache = {}


def _build(alpha: float, plain_ln: bool = False):
    nc = bacc.Bacc(
        "TRN2",
        target_bir_lowering=False,
        debug=False,
        enable_asserts=False,
        num_devices=B,
    )

    # All big inputs are host-packed into the exact SBUF tile layout so each
    # DMA is 128 partitions x >=2KB contiguous (128 descriptors instead of
    # 2048): the descriptor-generation cost on the issuing queue engine was
    # the round-1 bottleneck (3.4-5.2us per panel).
    qT_d = nc.dram_tensor("qT", [128, NB * L], F8, kind="ExternalInput")
    kT_d = nc.dram_tensor("kT", [128, NB * L], F8, kind="ExternalInput")
    vT_d = nc.dram_tensor("vT", [128, NB * L], F8, kind="ExternalInput")
    qres_d = nc.dram_tensor("qres", [L, D], F32, kind="ExternalInput")
    wqT_d = nc.dram_tensor("wqT", [8 * 128, NB * 256], F8, kind="ExternalInput")
    wkT_d = nc.dram_tensor("wkT", [8 * 128, NB * 256], F8, kind="ExternalInput")
    wvT_d = nc.dram_tensor("wvT", [8 * 128, NB * 256], F8, kind="ExternalInput")
    woT_d = nc.dram_tensor("woT", [8 * 128, 8 * 512], F8, kind="ExternalInput")
    csp_d = nc.dram_tensor("csp", [128, H * 2], F32, kind="ExternalInput")
    maskT_d = nc.dram_tensor("maskT", [128, LB * L], BF16, kind="ExternalInput")
    lng_d = nc.dram_tensor("lng", [D], F32, kind="ExternalInput")
    lnb_d = nc.dram_tensor("lnb", [D], F32, kind="ExternalInput")
    out_d = nc.dram_tensor("out", [L, D], F16, kind="ExternalOutput")

    with tile.TileContext(nc) as tc:
        _body(
            tc, alpha,
            qT_d, kT_d, vT_d, qres_d,
            wqT_d, wkT_d, wvT_d, woT_d,
            csp_d, maskT_d, lng_d, lnb_d, out_d,
            plain_ln,
        )
    nc.compile()
    return nc


def _body(tc, alpha, qT_d, kT_d, vT_d, qres_d, wqT_d, wkT_d, wvT_d, woT_d,
          csp_d, maskT_d, lng_d, lnb_d, out_d, plain_ln):
    nc = tc.nc

    with (
        tc.tile_pool(name="singles", bufs=1) as singles,
        tc.tile_pool(name="wpool", bufs=8) as wpool,
        tc.tile_pool(name="big", bufs=1) as big,
        tc.tile_pool(name="hgrp", bufs=2) as hgrp,
        tc.tile_pool(name="small", bufs=3) as small,
        # one shared 4-deep PSUM pool for projections + kv: their lifetimes
        # are mostly disjoint, so sharing slots doubles each phase's
        # pipelining depth within the same 8-bank budget
        tc.tile_pool(name="psA", bufs=4, space="PSUM") as psA,
        tc.tile_pool(name="an_ps", bufs=2, space="PSUM") as an_ps,
        tc.tile_pool(name="dv_ps", bufs=2, space="PSUM") as dv_ps,
    ):
        # ---- projections: XT[i, l] = sum_j WT[j, i] * xT[j, l] ----
        # K first (pk depends only on K), then V (kv + transposes), then Q.
        # Inputs stream on the gpsimd queue, weights on the sync queue, so
        # their issue costs overlap. The K input DMA goes first on gpsimd.
        xT_in = {}
        for name, dram in (("k", kT_d), ("v", vT_d), ("q", qT_d)):
            t = big.tile([128, NB, L], F8, tag=f"{name}T_in", name=f"{name}T_in")
            xT_in[name] = (t, dram)

        def load_xT(name):
            t, dram = xT_in[name]
            tf = t.rearrange("p n l -> p (n l)")
            # halves so the first j-blocks unblock matmuls sooner; the K
            # second half rides the sync queue ahead of the weight panels
            # (gpsimd's SWDGE drains delayed it ~5us behind the matmuls)
            nc.gpsimd.dma_start(out=tf[:, 0:8 * L], in_=dram.ap()[:, 0:8 * L])
            eng2 = nc.sync if name == "k" else nc.gpsimd
            eng2.dma_start(out=tf[:, 8 * L:NB * L], in_=dram.ap()[:, 8 * L:NB * L])

        load_xT("k")

        # constants after the K input on the gpsimd queue
        ident8 = singles.tile([128, 128], F8)
        make_identity(nc, ident8)
        # fp8 ones at 1/SA: den_ps = rowsum(am)/SA^2 with am = A/SA, so
        # rden = SA^2/den and attnT = num_ps*rden = SA*attn
        ones_t = singles.tile([128, 2, 128], F8)
        nc.vector.memset(ones_t, 1.0 / SA)
        eps_t = singles.tile([128, 1], F32)
        nc.vector.memset(eps_t, EPS)
        csp_t = singles.tile([128, H * 2], F32)
        nc.gpsimd.dma_start(out=csp_t, in_=csp_d.ap())

        # dummy matmuls while the first weight panels stream in: keeps the
        # PE-HAM activity monitor busy so the real stream starts at 2.4 GHz
        warm_ps = dv_ps.tile([128, 256], F32, tag="dv", name="warm_ps")
        for _ in range(24):
            nc.tensor.matmul(warm_ps[:, 0:128], ones_t[:, 0, :], ones_t[:, 0, :],
                             start=True, stop=True)

        w_ds = {"k": wkT_d, "v": wvT_d, "q": wqT_d, "o": woT_d}
        succ = {"k": "v", "v": "q", "q": "o"}
        prefetched = {}

        def panel_dma(name, iq, tag, halved=False):
            w_t = wpool.tile([128, NB, 256], F8, tag=tag, name=f"w_{name}{iq}")
            wf = w_t.rearrange("p n i -> p (n i)")
            rows = slice(iq * 128, (iq + 1) * 128)
            w_r = w_ds[name].ap()[rows, :]
            # alternate issue queues during the projections (ScalarE is idle
            # there) so issue latency and transfers overlap; outproj panels
            # stay on sync (ScalarE has real work by then)
            eng = nc.scalar if (name != "o" and iq % 2 == 1) else nc.sync
            if halved:
                eng.dma_start(out=wf[:, 0:8 * 256], in_=w_r[:, 0:8 * 256])
                eng.dma_start(out=wf[:, 8 * 256:NB * 256], in_=w_r[:, 8 * 256:NB * 256])
            else:
                eng.dma_start(out=wf, in_=w_r)
            return w_t

        # PSUM holds SW*X (fp8 weights are pre-scaled by SW on the host);
        # the copy to SBUF unscales — and folds alpha for Q. K/V land in fp8
        # (they only feed fp8 matmuls + phi); Q stays bf16 for the q-mod STT.
        # Copies run on ScalarE: DVE is the busier engine mid-kernel.
        unscale = {"k": 1.0 / SW, "v": 1.0 / SW, "q": alpha / SW}
        proj_dt = {"k": F8, "v": F8, "q": BF16}
        projs = {}
        for name in ("k", "v", "q"):
            out_t = big.tile([128, NB, L], proj_dt[name], tag=f"{name}proj",
                             name=f"{name}proj")
            x_t = xT_in[name][0]
            for iq in range(8):  # i-quarter: 2 output feature blocks
                w_t = prefetched.pop((name, iq), None)
                if w_t is None:
                    w_t = panel_dma(name, iq, "w", halved=(name == "k" and iq < 4))
                if iq == 3 and succ[name] != "o":
                    # next projection's activation streams during this proj
                    load_xT(succ[name])
                ps = psA.tile([128, 2, L], F32, tag="pk")
                for ib in range(2):
                    for j in range(0, NB, 2):  # DoubleRow: 2 k-blocks/matmul
                        nc.tensor.matmul(
                            ps[:, ib, :],
                            w_t[:, j:j + 2, ib * 128:(ib + 1) * 128],
                            x_t[:, j:j + 2, :],
                            start=(j == 0),
                            stop=(j == NB - 2),
                            perf_mode=DR,
                        )
                nc.scalar.activation(
                    out=out_t[:, iq * 2:iq * 2 + 2, :], in_=ps,
                    func=AF.Identity, scale=unscale[name],
                )
            projs[name] = out_t
        KT_t, VT_t, QT_t = projs["k"], projs["v"], projs["q"]

        maskT_t = singles.tile([128, LB, L], BF16)
        nc.gpsimd.dma_start(out=maskT_t.rearrange("p a l -> p (a l)"),
                            in_=maskT_d.ap())
        qres_t = []
        for lb in range(LB):
            t = big.tile([128, D], F32, tag=f"qres{lb}", name=f"qres{lb}")
            nc.gpsimd.dma_start(out=t, in_=qres_d.ap()[lb * 128:(lb + 1) * 128, :])
            qres_t.append(t)
        lng_t = lnb_t = None
        if not plain_ln:
            lng_t = singles.tile([128, D], F32)
            nc.gpsimd.dma_start(out=lng_t,
                                in_=lng_d.ap().partition_broadcast(128))
            lnb_t = singles.tile([128, D], F32)
            nc.gpsimd.dma_start(out=lnb_t,
                                in_=lnb_d.ap().partition_broadcast(128))

        # ---- pk = phi(KT) over all heads at once (fp8 out) ----
        pk_t = big.tile([128, NB, L], F8, tag="pk")
        ek_t = big.tile([128, NB, L], BF16, tag="ek")
        nc.vector.tensor_scalar_min(ek_t, KT_t, 0.0)
        nc.scalar.activation(ek_t, ek_t, AF.Exp)
        nc.vector.tensor_scalar_max(pk_t, KT_t, 0.0)
        nc.vector.tensor_add(pk_t, pk_t, ek_t)

        # ---- per-head-group (2 heads): kv + V-transpose + q-mod + phi(q) ----
        # V-transposes ride along per group so PE has filler work while the
        # group's phi chain runs on DVE/ACT.
        V_t = big.tile([128, LB, D], F8, tag="V")
        pq_t = big.tile([128, NB, L], F8, tag="pq")
        for g in range(4):  # groups of 2 heads
            kvm = hgrp.tile([128, 4, L], BF16, tag="kvm")
            for hh in range(2):
                h = 2 * g + hh
                n0 = 2 * h
                ps = psA.tile([128, 2, L], F32, tag="pk")
                for mb in range(2):
                    nc.tensor.matmul(
                        ps[:, mb, :],
                        VT_t[:, n0:n0 + 2, mb * 128:(mb + 1) * 128],
                        KT_t[:, n0:n0 + 2, :],
                        start=True, stop=True,
                        perf_mode=DR,
                    )
                for mb in range(2):
                    # q_mod = (alpha*Q) * (kv + cs*(1-alpha)/alpha); the
                    # alpha factor is folded into Wq on the host, so one STT
                    # straight from PSUM does modulate+multiply.
                    nc.vector.scalar_tensor_tensor(
                        out=kvm[:, 2 * hh + mb, :],
                        in0=ps[:, mb, :],
                        scalar=csp_t[:, n0 + mb:n0 + mb + 1],
                        in1=QT_t[:, n0 + mb, :],
                        op0=OP.add,
                        op1=OP.mult,
                    )
                for ib in range(LB):
                    # fp8 PE transpose writes 16-bit granules: the output AP
                    # must step by 2 elements; the copy below compacts it.
                    psv = dv_ps.tile([128, 512], F8, tag="dv")
                    for db in range(2):
                        nc.tensor.transpose(
                            psv[:, db * 256:(db + 1) * 256:2],
                            VT_t[:, n0 + db, ib * 128:(ib + 1) * 128],
                            ident8,
                        )
                    nc.scalar.copy(
                        out=V_t[:, ib, h * DH:h * DH + 256],
                        in_=psv[:, 0:512:2],
                    )
            # pq = phi(q_mod)
            qsl = slice(4 * g, 4 * g + 4)
            eq = hgrp.tile([128, 4, L], BF16, tag="eq")
            nc.vector.tensor_scalar_min(eq, kvm, 0.0)
            nc.scalar.activation(eq, eq, AF.Exp)
            nc.vector.tensor_scalar_max(kvm, kvm, 0.0)
            nc.vector.tensor_add(pq_t[:, qsl, :], eq, kvm)


        # ---- per-head: A matmul, mask, den, num, outT ----
        # attnT holds SA*attn in fp8 (|SA*attn| <= ~150 < 240 max)
        attnT_t = big.tile([128, NB, L], F8, tag="attnT")
        for h in range(H):
            n0 = 2 * h
            # causal block structure of AT[i, l] (i<=l kept):
            #   ib=0: l<128 lower-triangular, l>=128 all-ones
            #   ib=1: l<128 all-zero (skipped entirely), l>=128 triangular
            a_ps = an_ps.tile([128, 2, L], F32, tag="an")
            nc.tensor.matmul(
                a_ps[:, 0, :],
                pk_t[:, n0:n0 + 2, 0:128],
                pq_t[:, n0:n0 + 2, :],
                start=True, stop=True, perf_mode=DR,
            )
            nc.tensor.matmul(
                a_ps[:, 1, 128:L],
                pk_t[:, n0:n0 + 2, 128:L],
                pq_t[:, n0:n0 + 2, 128:L],
                start=True, stop=True, perf_mode=DR,
            )
            # am = A/SA in fp8 (maskT holds 1/SA); the always-unmasked
            # middle block gets the 1/SA scale on ScalarE
            am = small.tile([128, LB, L], F8, tag="am")
            nc.vector.tensor_mul(am[:, 0, 0:128], a_ps[:, 0, 0:128],
                                 maskT_t[:, 0, 0:128])
            nc.scalar.activation(out=am[:, 0, 128:L], in_=a_ps[:, 0, 128:L],
                                 func=AF.Identity, scale=1.0 / SA)
            nc.vector.tensor_mul(am[:, 1, 128:L], a_ps[:, 1, 128:L],
                                 maskT_t[:, 1, 128:L])

            den_ps = dv_ps.tile([128, L], F32, tag="dv", name="den_ps")
            nc.tensor.matmul(den_ps[:, 0:128], ones_t[:, 0, :], am[:, 0, 0:128],
                             start=True, stop=True)
            nc.tensor.matmul(den_ps[:, 128:L], ones_t, am[:, 0:2, 128:L],
                             start=True, stop=True, perf_mode=DR)
            # den is a sum of strictly positive phi-products (>= O(0.01)
            # mathematically, O(100) in practice), so the reference's 1e-8
            # clamp can never bind — reciprocal straight from PSUM.
            rden = small.tile([128, L], F32, tag="rden")
            nc.vector.reciprocal_approx_fast(out=rden, in_=den_ps)

            n_ps = an_ps.tile([128, 2, L], F32, tag="an")
            for db in range(2):
                dsl = slice(h * DH + db * 128, h * DH + (db + 1) * 128)
                nc.tensor.matmul(n_ps[:, db, 0:128], V_t[:, 0, dsl],
                                 am[:, 0, 0:128], start=True, stop=True)
                nc.tensor.matmul(n_ps[:, db, 128:L], V_t[:, 0:2, dsl],
                                 am[:, 0:2, 128:L],
                                 start=True, stop=True, perf_mode=DR)
            for db in range(2):
                nc.vector.tensor_mul(attnT_t[:, n0 + db, :], n_ps[:, db, :], rden)

        # Trigger the sqrt ACT-table load now — after ScalarE's last
        # Copy/Exp user, off the LN tail's critical path (the set switch
        # costs ~2.6us).
        warm_sqrt = singles.tile([128, 1], F32)
        nc.scalar.activation(warm_sqrt, eps_t, AF.Sqrt)

        # ---- output projection + residual + LayerNorm ----
        # 512-wide moving operand: half the matmul and LDWEIGHTS count of
        # the input projections. All 8 panels are preloaded (wpool bufs=8)
        # so the loop can run lb-major: block 0's LayerNorm overlaps block
        # 1's matmuls instead of serializing at the tail.
        x_sb = [big.tile([128, D], F32, tag=f"x{lb}", name=f"x{lb}")
                for lb in range(LB)]
        stats = [small.tile([128, 4, 6], F32, tag=f"stats{lb}",
                            name=f"stats{lb}", bufs=1) for lb in range(LB)]
        wo_tiles = []
        for nq in range(4):
            wo = []
            for jh in range(2):
                w_t = wpool.tile([128, 8, 512], F8, tag="w",
                                 name=f"w_o{nq}{jh}")
                rows = slice((nq * 2 + jh) * 128, (nq * 2 + jh + 1) * 128)
                nc.sync.dma_start(
                    out=w_t.rearrange("p n i -> p (n i)"),
                    in_=w_ds["o"].ap()[rows, :])
                wo.append(w_t)
            wo_tiles.append(wo)

        def outproj_block(lb):
            for nq in range(4):
                wo = wo_tiles[nq]
                ps = psA.tile([128, 2, L], F32, tag="pk")
                psf = ps.rearrange("p a l -> p (a l)")
                for j in range(0, NB, 2):  # DoubleRow k-block pairs
                    nc.tensor.matmul(
                        psf,
                        attnT_t[:, j:j + 2, lb * 128:(lb + 1) * 128],
                        wo[j // 8][:, j % 8:j % 8 + 2, :],
                        start=(j == 0),
                        stop=(j == NB - 2),
                        perf_mode=DR,
                    )
                # x = o + (query + bo)
                sl = slice(nq * 512, (nq + 1) * 512)
                nc.vector.tensor_add(x_sb[lb][:, sl], psf, qres_t[lb][:, sl])
                # LN stats pipelined per 512-chunk while later chunks project
                nc.vector.bn_stats(out=stats[lb][:, nq, :],
                                   in_=x_sb[lb][:, sl])

        def ln_block(lb):
            x = x_sb[lb]
            x16 = big.tile([128, D], F16, tag=f"x16{lb}", name=f"x16{lb}")
            mv = small.tile([128, 2], F32, tag="mv")
            nc.vector.bn_aggr(out=mv, in_=stats[lb])
            sd = small.tile([128, 1], F32, tag="sd")
            nc.scalar.activation(sd, mv[:, 1:2], AF.Sqrt, bias=eps_t)
            nc.vector.reciprocal_approx_fast(out=sd, in_=sd)
            nsdmu = small.tile([128, 1], F32, tag="nsdmu")
            nc.vector.tensor_scalar(
                out=nsdmu, in0=sd, scalar1=mv[:, 0:1], scalar2=-1.0,
                op0=OP.mult, op1=OP.mult,
            )
            for ch in range(4):  # quarters, so DVE work overlaps output DMA
                sl = slice(ch * (D // 4), (ch + 1) * (D // 4))
                if plain_ln:
                    # ln_g == 1, ln_b == 0: fused (x - mu) * rstd, split
                    # across DVE and the idle ScalarE (as rstd*x - rstd*mu)
                    if ch % 2 == 0:
                        nc.vector.tensor_scalar(
                            out=x16[:, sl], in0=x[:, sl], scalar1=mv[:, 0:1],
                            scalar2=sd, op0=OP.subtract, op1=OP.mult,
                        )
                    else:
                        nc.scalar.activation(
                            out=x16[:, sl], in_=x[:, sl], func=AF.Identity,
                            bias=nsdmu, scale=sd,
                        )
                else:
                    nc.vector.tensor_scalar(
                        out=x[:, sl], in0=x[:, sl], scalar1=mv[:, 0:1],
                        scalar2=None, op0=OP.subtract,
                    )
                    nc.vector.scalar_tensor_tensor(
                        out=x[:, sl], in0=x[:, sl], scalar=sd, in1=lng_t[:, sl],
                        op0=OP.mult, op1=OP.mult,
                    )
                    nc.vector.tensor_add(x16[:, sl], x[:, sl], lnb_t[:, sl])
                # alternate output-DMA issue queues so the ~1.2us issue
                # costs overlap at the tail
                oeng = nc.sync if ch % 2 == 0 else nc.gpsimd
                oeng.dma_start(
                    out=out_d.ap()[lb * 128:(lb + 1) * 128, sl], in_=x16[:, sl])

        for lb in range(LB):
            outproj_block(lb)
            ln_block(lb)


def _host_prep(query, key, value, Wq, Wk, Wv, Wo, bo, ln_g, ln_b, alpha, beta):
    """Host-side: cumulative_state shortcut + layout/dtype marshaling."""
    a, b = float(alpha), float(beta)
    f64 = np.float64
    # mean over (batch, l) of kv[b,h,l,m] = (1/(B*L)) sum_b Ksum[b,h,:].V[b,h,m,:]
    keysum = key.astype(f64).sum(axis=1)                      # [B, D]
    Ksum = (keysum @ Wk.T.astype(f64)).reshape(B, H, DH)      # [B, H, DH]
    WvH = Wv.astype(f64).reshape(H, DH, D)
    wv_eff = np.einsum("hdj,bhd->bhj", WvH, Ksum, optimize=True)      # [B,H,D]
    contrib = np.einsum("bmj,bhj->hm", value.astype(f64), wv_eff, optimize=True)
    mean_kv = contrib / (B * L)                               # [H, DH]
    cs = np.zeros((H, DH), f64)
    c = np.zeros(DH, f64)
    for h in range(H):
        cs[h] = c
        c = b * c + a * mean_kv[h]
    # q_mod = Q*((1-a)*cs + a*kv) = (a*Q)*(kv + (1-a)/a*cs); a is folded
    # into the Q PSUM-copy scale on device, and this is cs*(1-a)/a:
    csp = ((1.0 - a) / a * cs if a != 0 else 0.0 * cs).astype(np.float32)
    csp_dev = np.ascontiguousarray(
        csp.reshape(H, 2, 128).transpose(2, 0, 1).reshape(128, H * 2)
    )
    plain_ln = bool(np.all(ln_g == 1.0) and np.all(ln_b == 0.0))

    bf = ml_dtypes.bfloat16
    f8 = ml_dtypes.float8_e4m3  # TRN fp8e4: max 240, matches bit-for-bit

    def to8(x):
        return np.clip(x, -240.0, 240.0).astype(f8)

    # pack into the exact SBUF tile layouts (one contiguous run per
    # partition per DMA): proj panels [iq*128+p, n*256+c] = wT[n*128+p,
    # iq*256+c]; outproj [(nq*2+jh)*128+p, k*512+c] = woT[(jh*8+k)*128+p,
    # nq*512+c]; activations [p, n*L+l] = xT[n*128+p, l].
    def pack_w(wT):
        arr = np.asarray(wT).reshape(NB, 128, 8, 256)
        return np.ascontiguousarray(
            arr.transpose(2, 1, 0, 3).reshape(8 * 128, NB * 256))

    def pack_wo(woT_):
        arr = np.asarray(woT_).reshape(2, 8, 128, 4, 512)
        return np.ascontiguousarray(
            arr.transpose(3, 0, 2, 1, 4).reshape(8 * 128, 8 * 512))

    def pack_x(xT):
        arr = np.asarray(xT).reshape(NB, 128, L)
        return np.ascontiguousarray(arr.transpose(1, 0, 2).reshape(128, NB * L))

    qT = np.stack([pack_x(to8(query[c].T)) for c in range(B)])
    kT = np.stack([pack_x(to8(key[c].T)) for c in range(B)])
    vT = np.stack([pack_x(to8(value[c].T)) for c in range(B)])
    wqT = pack_w(to8(SW * Wq.T))
    wkT = pack_w(to8(SW * Wk.T))
    wvT = pack_w(to8(SW * Wv.T))
    woT = pack_wo(to8(SW * Wo.T))
    # out-proj PSUM is SW*SA*o; scaling the residual to match makes
    # x_dev = SW*SA*x, and LayerNorm is invariant to uniform scaling.
    qres = (SW * SA * (query + bo[None, None, :])).astype(np.float32)
    # mask[i,l] = 1/SA iff i<=l: folds the fp8 am = A/SA scale into the mask
    mask_full = np.triu(np.full((L, L), 1.0 / SA, np.float32))
    maskT = np.ascontiguousarray(
        mask_full.reshape(LB, 128, L).transpose(1, 0, 2).reshape(128, LB * L)
    ).astype(bf)

    in_maps = []
    for c_ in range(B):
        in_maps.append({
            "qT": qT[c_], "kT": kT[c_], "vT": vT[c_],
            "qres": qres[c_],
            "wqT": wqT, "wkT": wkT, "wvT": wvT, "woT": woT,
            "csp": csp_dev, "maskT": maskT,
            "lng": ln_g.astype(np.float32), "lnb": ln_b.astype(np.float32),
        })
    return in_maps, a, plain_ln


def get_nc(alpha: float, plain_ln: bool = True):
    key = (round(float(alpha), 9), bool(plain_ln))
    if key not in _cache:
        _cache[key] = _build(float(alpha), bool(plain_ln))
    return _cache[key]


def kernel(query, key, value, Wq, Wk, Wv, Wo, bo, ln_g, ln_b, alpha, beta,
           _trace=False, _trace_kwargs=None):
    args = [np.asarray(x) for x in
            (query, key, value, Wq, Wk, Wv, Wo, bo, ln_g, ln_b, alpha, beta)]
    in_maps, a, plain_ln = _host_prep(*args)
    nc = get_nc(a, plain_ln)
    res = run_bass_kernel_spmd(
        nc, in_maps, core_ids=list(range(B)),
        trace=_trace, **(_trace_kwargs or {}),
    )
    out = np.stack([res.results[c]["out"] for c in range(B)], axis=0)
    out = out.astype(np.float32)
    if _trace:
        kernel._last_results = res
    return out



# revision 43
# speedup vs baseline: 1.1353x; 1.1353x over previous
"""DeltaNet attention TRN2 kernel (nn_DeltaNetAttention_5299989643476).

Strategy: data-parallel over batch (8 batches -> 8 NeuronCores). The
cross-batch cumulative_state scan is tiny ([H, Dh]) and is computed on the
host via an algebraic shortcut (mean over (b,l) of kv == Ksum . V
contraction), then passed to every core as a small constant tensor, so the
device program needs no collectives.

On-device, everything runs in a "transposed" layout (features on
partitions, sequence on the free dim):
  - QT/KT/VT projections: weight-stationary fp8e4 DoubleRow matmuls (2x PE
    throughput), fp32 PSUM accum; weights host-prescaled by SW=64 to clear
    the fp8 denormal range, unscaled in the PSUM->SBUF copy
  - per head: kvT matmul; q-mod via tensor_scalar with per-partition cs;
    phi(x)=elu(x)+1 = relu(x)+exp(min(x,0)); causal linear attention as a
    masked A=pq@pk^T matmul; den via an all-ones stationary matmul (which
    also replicates den across partitions for the division broadcast);
    num needs V back in sequence-major layout -> PE transpose
  - output projection + residual + LayerNorm (bn_stats/bn_aggr)
"""

import numpy as np
import ml_dtypes

import concourse.bass as bass
import concourse.mybir as mybir
import concourse.tile as tile
from concourse import bacc
from concourse.bass_utils import run_bass_kernel_spmd
from concourse.masks import make_identity


def _ensure_axon_hooks():
    """This image's `antenv` lacks `axon_hooks`; if the caller's environment
    sets BASS_TRACE, run_bass_kernel_spmd would crash importing it. Register
    a no-op shim (only when absent) so tracing degrades gracefully."""
    try:
        import antenv.axon_hooks  # noqa: F401
    except ImportError:
        import sys
        import types

        import antenv

        mod = types.ModuleType("antenv.axon_hooks")
        _h = [None]
        mod.set_axon_ntff_profile_hook = lambda h: _h.__setitem__(0, h)
        mod.get_axon_ntff_profile_hook = lambda: _h[0]
        sys.modules["antenv.axon_hooks"] = mod
        antenv.axon_hooks = mod


_ensure_axon_hooks()

B, L, D, H = 8, 256, 2048, 8
DH = D // H            # 256
NB = D // 128          # 16 feature blocks of 128
LB = L // 128          # 2 sequence blocks of 128
EPS = 1e-5
SW = 64.0              # fp8 weight pre-scale (power of 2: exact)
SA = 32.0              # fp8 attn pre-scale; SW*SA is folded into qres

F32 = mybir.dt.float32
F16 = mybir.dt.float16
BF16 = mybir.dt.bfloat16
F8 = mybir.dt.float8e4
AF = mybir.ActivationFunctionType
OP = mybir.AluOpType
DR = mybir.MatmulPerfMode.DoubleRow

_cache = {}


def _build(alpha: float, plain_ln: bool = False):
    nc = bacc.Bacc(
        "TRN2",
        target_bir_lowering=False,
        debug=False,
        enable_asserts=False,
        num_devices=B,
    )

    # All big inputs are host-packed into the exact SBUF tile layout so each
    # DMA is 128 partitions x >=2KB contiguous (128 descriptors instead of
    # 2048): the descriptor-generation cost on the issuing queue engine was
    # the round-1 bottleneck (3.4-5.2us per panel).
    qT_d = nc.dram_tensor("qT", [128, NB * L], F8, kind="ExternalInput")
    kT_d = nc.dram_tensor("kT", [128, NB * L], F8, kind="ExternalInput")
    vT_d = nc.dram_tensor("vT", [128, NB * L], F8, kind="ExternalInput")
    qres_d = nc.dram_tensor("qres", [L, D], F16, kind="ExternalInput")
    wqT_d = nc.dram_tensor("wqT", [8 * 128, NB * 256], F8, kind="ExternalInput")
    wkT_d = nc.dram_tensor("wkT", [8 * 128, NB * 256], F8, kind="ExternalInput")
    wvT_d = nc.dram_tensor("wvT", [8 * 128, NB * 256], F8, kind="ExternalInput")
    woT_d = nc.dram_tensor("woT", [8 * 128, 8 * 512], F8, kind="ExternalInput")
    csp_d = nc.dram_tensor("csp", [128, H * 2], F32, kind="ExternalInput")
    maskT_d = nc.dram_tensor("maskT", [128, LB * L], BF16, kind="ExternalInput")
    lng_d = nc.dram_tensor("lng", [D], F32, kind="ExternalInput")
    lnb_d = nc.dram_tensor("lnb", [D], F32, kind="ExternalInput")
    out_d = nc.dram_tensor("out", [L, D], F16, kind="ExternalOutput")

    with tile.TileContext(nc) as tc:
        _body(
            tc, alpha,
            qT_d, kT_d, vT_d, qres_d,
            wqT_d, wkT_d, wvT_d, woT_d,
            csp_d, maskT_d, lng_d, lnb_d, out_d,
            plain_ln,
        )
    nc.compile()
    return nc


def _body(tc, alpha, qT_d, kT_d, vT_d, qres_d, wqT_d, wkT_d, wvT_d, woT_d,
          csp_d, maskT_d, lng_d, lnb_d, out_d, plain_ln):
    nc = tc.nc

    with (
        tc.tile_pool(name="singles", bufs=1) as singles,
        tc.tile_pool(name="wpool", bufs=8) as wpool,
        tc.tile_pool(name="big", bufs=1) as big,
        tc.tile_pool(name="hgrp", bufs=2) as hgrp,
        tc.tile_pool(name="small", bufs=3) as small,
        # one shared 4-deep PSUM pool for projections + kv: their lifetimes
        # are mostly disjoint, so sharing slots doubles each phase's
        # pipelining depth within the same 8-bank budget
        tc.tile_pool(name="psA", bufs=4, space="PSUM") as psA,
        tc.tile_pool(name="an_ps", bufs=2, space="PSUM") as an_ps,
        tc.tile_pool(name="dv_ps", bufs=2, space="PSUM") as dv_ps,
    ):
        # ---- projections: XT[i, l] = sum_j WT[j, i] * xT[j, l] ----
        # K first (pk depends only on K), then V (kv + transposes), then Q.
        # Inputs stream on the gpsimd queue, weights on the sync queue, so
        # their issue costs overlap. The K input DMA goes first on gpsimd.
        xT_in = {}
        for name, dram in (("k", kT_d), ("v", vT_d), ("q", qT_d)):
            t = big.tile([128, NB, L], F8, tag=f"{name}T_in", name=f"{name}T_in")
            xT_in[name] = (t, dram)

        def load_xT(name):
            t, dram = xT_in[name]
            tf = t.rearrange("p n l -> p (n l)")
            # halves so the first j-blocks unblock matmuls sooner. K loads at
            # t=0 when DVE is idle, so gpsimd's SWDGE is safe; the V/Q loads
            # happen while DVE runs fp32 PSUM copies, which lock the shared
            # SBUF port and starve SWDGE - route those via HWDGE (scalar).
            eng1 = nc.gpsimd if name == "k" else nc.scalar
            eng2 = nc.sync if name == "k" else nc.scalar
            eng1.dma_start(out=tf[:, 0:8 * L], in_=dram.ap()[:, 0:8 * L])
            eng2.dma_start(out=tf[:, 8 * L:NB * L], in_=dram.ap()[:, 8 * L:NB * L])

        load_xT("k")

        # constants after the K input on the gpsimd queue
        ident8 = singles.tile([128, 128], F8)
        make_identity(nc, ident8)
        # fp8 ones at 1/SA: den_ps = rowsum(am)/SA^2 with am = A/SA, so
        # rden = SA^2/den and attnT = num_ps*rden = SA*attn
        ones_t = singles.tile([128, 2, 128], F8)
        nc.vector.memset(ones_t, 1.0 / SA)
        eps_t = singles.tile([128, 1], F32)
        nc.vector.memset(eps_t, EPS)
        csp_t = singles.tile([128, H * 2], F32)
        nc.gpsimd.dma_start(out=csp_t, in_=csp_d.ap())

        # dummy matmuls while the first weight panels stream in: keeps the
        # PE-HAM activity monitor busy so the real stream starts at 2.4 GHz
        warm_ps = dv_ps.tile([128, 256], F32, tag="dv", name="warm_ps")
        for _ in range(24):
            nc.tensor.matmul(warm_ps[:, 0:128], ones_t[:, 0, :], ones_t[:, 0, :],
                             start=True, stop=True)

        w_ds = {"k": wkT_d, "v": wvT_d, "q": wqT_d, "o": woT_d}
        succ = {"k": "v", "v": "q", "q": "o"}
        prefetched = {}

        def panel_dma(name, iq, tag, halved=False):
            w_t = wpool.tile([128, NB, 256], F8, tag=tag, name=f"w_{name}{iq}")
            wf = w_t.rearrange("p n i -> p (n i)")
            rows = slice(iq * 128, (iq + 1) * 128)
            w_r = w_ds[name].ap()[rows, :]
            # alternate issue queues during the projections (ScalarE is idle
            # there) so issue latency and transfers overlap; outproj panels
            # stay on sync (ScalarE has real work by then)
            eng = nc.scalar if (name != "o" and iq % 2 == 1) else nc.sync
            if halved:
                eng.dma_start(out=wf[:, 0:8 * 256], in_=w_r[:, 0:8 * 256])
                eng.dma_start(out=wf[:, 8 * 256:NB * 256], in_=w_r[:, 8 * 256:NB * 256])
            else:
                eng.dma_start(out=wf, in_=w_r)
            return w_t

        # PSUM holds SW*X (fp8 weights are pre-scaled by SW on the host);
        # the copy to SBUF unscales — and folds alpha for Q. K/V land in fp8
        # (they only feed fp8 matmuls + phi); Q stays bf16 for the q-mod STT.
        # Copies run on ScalarE: DVE is the busier engine mid-kernel.
        unscale = {"k": 1.0 / SW, "v": 1.0 / SW, "q": alpha / SW}
        proj_dt = {"k": F8, "v": F8, "q": BF16}
        projs = {}
        for name in ("k", "v", "q"):
            out_t = big.tile([128, NB, L], proj_dt[name], tag=f"{name}proj",
                             name=f"{name}proj")
            x_t = xT_in[name][0]
            for iq in range(8):  # i-quarter: 2 output feature blocks
                w_t = prefetched.pop((name, iq), None)
                if w_t is None:
                    w_t = panel_dma(name, iq, "w", halved=(name == "k" and iq < 4))
                if iq == 3 and succ[name] != "o":
                    # next projection's activation streams during this proj
                    load_xT(succ[name])
                ps = psA.tile([128, 2, L], F32, tag="pk")
                for ib in range(2):
                    for j in range(0, NB, 2):  # DoubleRow: 2 k-blocks/matmul
                        nc.tensor.matmul(
                            ps[:, ib, :],
                            w_t[:, j:j + 2, ib * 128:(ib + 1) * 128],
                            x_t[:, j:j + 2, :],
                            start=(j == 0),
                            stop=(j == NB - 2),
                            perf_mode=DR,
                        )
                nc.vector.tensor_scalar(
                    out=out_t[:, iq * 2:iq * 2 + 2, :], in0=ps,
                    scalar1=unscale[name], scalar2=None, op0=OP.mult,
                )
            projs[name] = out_t
        KT_t, VT_t, QT_t = projs["k"], projs["v"], projs["q"]

        maskT_t = singles.tile([128, LB, L], BF16)
        nc.gpsimd.dma_start(out=maskT_t.rearrange("p a l -> p (a l)"),
                            in_=maskT_d.ap())
        qres_t = []
        for lb in range(LB):
            t = big.tile([128, D], F16, tag=f"qres{lb}", name=f"qres{lb}")
            nc.gpsimd.dma_start(out=t, in_=qres_d.ap()[lb * 128:(lb + 1) * 128, :])
            qres_t.append(t)
        lng_t = lnb_t = None
        if not plain_ln:
            lng_t = singles.tile([128, D], F32)
            nc.gpsimd.dma_start(out=lng_t,
                                in_=lng_d.ap().partition_broadcast(128))
            lnb_t = singles.tile([128, D], F32)
            nc.gpsimd.dma_start(out=lnb_t,
                                in_=lnb_d.ap().partition_broadcast(128))

        # ---- pk = phi(KT) over all heads at once (fp8 out) ----
        # phi(x) = max(x,0) + exp(min(x,0)) in 2 DVE ops + 1 ACT op: the
        # max+add collapse into one STT.
        pk_t = big.tile([128, NB, L], F8, tag="pk")
        ek_t = big.tile([128, NB, L], BF16, tag="ek")
        nc.vector.tensor_scalar_min(ek_t, KT_t, 0.0)
        nc.scalar.activation(ek_t, ek_t, AF.Exp)
        nc.vector.scalar_tensor_tensor(
            out=pk_t, in0=KT_t, scalar=0.0, in1=ek_t,
            op0=OP.max, op1=OP.add,
        )

        # ---- per-head-group (2 heads): kv + V-transpose + q-mod + phi(q) ----
        # V-transposes ride along per group so PE has filler work while the
        # group's phi chain runs on DVE/ACT.
        V_t = big.tile([128, LB, D], F8, tag="V")
        pq_t = big.tile([128, NB, L], F8, tag="pq")
        for g in range(4):  # groups of 2 heads
            kvm = hgrp.tile([128, 4, L], BF16, tag="kvm")
            for hh in range(2):
                h = 2 * g + hh
                n0 = 2 * h
                ps = psA.tile([128, 2, L], F32, tag="pk")
                for mb in range(2):
                    nc.tensor.matmul(
                        ps[:, mb, :],
                        VT_t[:, n0:n0 + 2, mb * 128:(mb + 1) * 128],
                        KT_t[:, n0:n0 + 2, :],
                        start=True, stop=True,
                        perf_mode=DR,
                    )
                for mb in range(2):
                    # q_mod = (alpha*Q) * (kv + cs*(1-alpha)/alpha); the
                    # alpha factor is folded into Wq on the host, so one STT
                    # straight from PSUM does modulate+multiply.
                    nc.vector.scalar_tensor_tensor(
                        out=kvm[:, 2 * hh + mb, :],
                        in0=ps[:, mb, :],
                        scalar=csp_t[:, n0 + mb:n0 + mb + 1],
                        in1=QT_t[:, n0 + mb, :],
                        op0=OP.add,
                        op1=OP.mult,
                    )
                for ib in range(LB):
                    # fp8 PE transpose writes 16-bit granules: the output AP
                    # must step by 2 elements; the copy below compacts it.
                    psv = dv_ps.tile([128, 512], F8, tag="dv")
                    for db in range(2):
                        nc.tensor.transpose(
                            psv[:, db * 256:(db + 1) * 256:2],
                            VT_t[:, n0 + db, ib * 128:(ib + 1) * 128],
                            ident8,
                        )
                    nc.scalar.copy(
                        out=V_t[:, ib, h * DH:h * DH + 256],
                        in_=psv[:, 0:512:2],
                    )
            # pq = phi(q_mod): min -> Exp -> fused max+add STT
            qsl = slice(4 * g, 4 * g + 4)
            eq = hgrp.tile([128, 4, L], BF16, tag="eq")
            nc.vector.tensor_scalar_min(eq, kvm, 0.0)
            nc.scalar.activation(eq, eq, AF.Exp)
            nc.vector.scalar_tensor_tensor(
                out=pq_t[:, qsl, :], in0=kvm, scalar=0.0, in1=eq,
                op0=OP.max, op1=OP.add,
            )


        # ---- per-head: A matmul, mask, den, num, outT ----
        # attnT holds SA*attn in fp8 (|SA*attn| <= ~150 < 240 max)
        attnT_t = big.tile([128, NB, L], F8, tag="attnT")
        for h in range(H):
            n0 = 2 * h
            # causal block structure of AT[i, l] (i<=l kept):
            #   ib=0: l<128 lower-triangular, l>=128 all-ones
            #   ib=1: l<128 all-zero (skipped entirely), l>=128 triangular
            a_ps = an_ps.tile([128, 2, L], F32, tag="an")
            nc.tensor.matmul(
                a_ps[:, 0, :],
                pk_t[:, n0:n0 + 2, 0:128],
                pq_t[:, n0:n0 + 2, :],
                start=True, stop=True, perf_mode=DR,
            )
            nc.tensor.matmul(
                a_ps[:, 1, 128:L],
                pk_t[:, n0:n0 + 2, 128:L],
                pq_t[:, n0:n0 + 2, 128:L],
                start=True, stop=True, perf_mode=DR,
            )
            # am = A/SA in fp8 (maskT holds 1/SA); the always-unmasked
            # middle block gets the 1/SA scale on ScalarE
            am = small.tile([128, LB, L], F8, tag="am")
            nc.vector.tensor_mul(am[:, 0, 0:128], a_ps[:, 0, 0:128],
                                 maskT_t[:, 0, 0:128])
            nc.scalar.activation(out=am[:, 0, 128:L], in_=a_ps[:, 0, 128:L],
                                 func=AF.Identity, scale=1.0 / SA)
            nc.vector.tensor_mul(am[:, 1, 128:L], a_ps[:, 1, 128:L],
                                 maskT_t[:, 1, 128:L])

            den_ps = dv_ps.tile([128, L], F32, tag="dv", name="den_ps")
            nc.tensor.matmul(den_ps[:, 0:128], ones_t[:, 0, :], am[:, 0, 0:128],
                             start=True, stop=True)
            nc.tensor.matmul(den_ps[:, 128:L], ones_t, am[:, 0:2, 128:L],
                             start=True, stop=True, perf_mode=DR)
            # den is a sum of strictly positive phi-products (>= O(0.01)
            # mathematically, O(100) in practice), so the reference's 1e-8
            # clamp can never bind — reciprocal straight from PSUM.
            rden = small.tile([128, L], F32, tag="rden")
            nc.vector.reciprocal_approx_fast(out=rden, in_=den_ps)

            n_ps = an_ps.tile([128, 2, L], F32, tag="an")
            for db in range(2):
                dsl = slice(h * DH + db * 128, h * DH + (db + 1) * 128)
                nc.tensor.matmul(n_ps[:, db, 0:128], V_t[:, 0, dsl],
                                 am[:, 0, 0:128], start=True, stop=True)
                nc.tensor.matmul(n_ps[:, db, 128:L], V_t[:, 0:2, dsl],
                                 am[:, 0:2, 128:L],
                                 start=True, stop=True, perf_mode=DR)
            for db in range(2):
                nc.vector.tensor_mul(attnT_t[:, n0 + db, :], n_ps[:, db, :], rden)

        # Trigger the sqrt ACT-table load now — after ScalarE's last
        # Copy/Exp user, off the LN tail's critical path (the set switch
        # costs ~2.6us).
        warm_sqrt = singles.tile([128, 1], F32)
        nc.scalar.activation(warm_sqrt, eps_t, AF.Sqrt)

        # ---- output projection + residual + LayerNorm ----
        # 512-wide moving operand: half the matmul and LDWEIGHTS count of
        # the input projections. All 8 panels are preloaded (wpool bufs=8)
        # so the loop can run lb-major: block 0's LayerNorm overlaps block
        # 1's matmuls instead of serializing at the tail.
        x_sb = [big.tile([128, D], F32, tag=f"x{lb}", name=f"x{lb}")
                for lb in range(LB)]
        stats = [small.tile([128, 4, 6], F32, tag=f"stats{lb}",
                            name=f"stats{lb}", bufs=1) for lb in range(LB)]
        wo_tiles = []
        for nq in range(4):
            wo = []
            for jh in range(2):
                w_t = wpool.tile([128, 8, 512], F8, tag="w",
                                 name=f"w_o{nq}{jh}")
                rows = slice((nq * 2 + jh) * 128, (nq * 2 + jh + 1) * 128)
                nc.sync.dma_start(
                    out=w_t.rearrange("p n i -> p (n i)"),
                    in_=w_ds["o"].ap()[rows, :])
                wo.append(w_t)
            wo_tiles.append(wo)

        def outproj_block(lb):
            for nq in range(4):
                wo = wo_tiles[nq]
                ps = psA.tile([128, 2, L], F32, tag="pk")
                psf = ps.rearrange("p a l -> p (a l)")
                for j in range(0, NB, 2):  # DoubleRow k-block pairs
                    nc.tensor.matmul(
                        psf,
                        attnT_t[:, j:j + 2, lb * 128:(lb + 1) * 128],
                        wo[j // 8][:, j % 8:j % 8 + 2, :],
                        start=(j == 0),
                        stop=(j == NB - 2),
                        perf_mode=DR,
                    )
                # x = o + (query + bo)
                sl = slice(nq * 512, (nq + 1) * 512)
                nc.vector.tensor_add(x_sb[lb][:, sl], psf, qres_t[lb][:, sl])
                # LN stats pipelined per 512-chunk while later chunks project
                nc.vector.bn_stats(out=stats[lb][:, nq, :],
                                   in_=x_sb[lb][:, sl])

        def ln_block(lb):
            x = x_sb[lb]
            x16 = big.tile([128, D], F16, tag=f"x16{lb}", name=f"x16{lb}")
            mv = small.tile([128, 2], F32, tag="mv")
            nc.vector.bn_aggr(out=mv, in_=stats[lb])
            sd = small.tile([128, 1], F32, tag="sd")
            nc.scalar.activation(sd, mv[:, 1:2], AF.Sqrt, bias=eps_t)
            nc.vector.reciprocal_approx_fast(out=sd, in_=sd)
            nsdmu = small.tile([128, 1], F32, tag="nsdmu")
            nc.vector.tensor_scalar(
                out=nsdmu, in0=sd, scalar1=mv[:, 0:1], scalar2=-1.0,
                op0=OP.mult, op1=OP.mult,
            )
            for ch in range(4):  # quarters, so DVE work overlaps output DMA
                sl = slice(ch * (D // 4), (ch + 1) * (D // 4))
                if plain_ln:
                    # ln_g == 1, ln_b == 0: fused (x - mu) * rstd, split
                    # across DVE and the idle ScalarE (as rstd*x - rstd*mu)
                    if ch % 2 == 0:
                        nc.vector.tensor_scalar(
                            out=x16[:, sl], in0=x[:, sl], scalar1=mv[:, 0:1],
                            scalar2=sd, op0=OP.subtract, op1=OP.mult,
                        )
                    else:
                        nc.scalar.activation(
                            out=x16[:, sl], in_=x[:, sl], func=AF.Identity,
                            bias=nsdmu, scale=sd,
                        )
                else:
                    nc.vector.tensor_scalar(
                        out=x[:, sl], in0=x[:, sl], scalar1=mv[:, 0:1],
                        scalar2=None, op0=OP.subtract,
                    )
                    nc.vector.scalar_tensor_tensor(
                        out=x[:, sl], in0=x[:, sl], scalar=sd, in1=lng_t[:, sl],
                        op0=OP.mult, op1=OP.mult,
                    )
                    nc.vector.tensor_add(x16[:, sl], x[:, sl], lnb_t[:, sl])
                # alternate output-DMA issue queues (both HWDGE: gpsimd's
                # SWDGE would starve behind the DVE LN ops here)
                oeng = nc.sync if ch % 2 == 0 else nc.scalar
                oeng.dma_start(
                    out=out_d.ap()[lb * 128:(lb + 1) * 128, sl], in_=x16[:, sl])

        for lb in range(LB):
            outproj_block(lb)
            ln_block(lb)


def _host_prep(query, key, value, Wq, Wk, Wv, Wo, bo, ln_g, ln_b, alpha, beta):
    """Host-side: cumulative_state shortcut + layout/dtype marshaling."""
    a, b = float(alpha), float(beta)
    f64 = np.float64
    # mean over (batch, l) of kv[b,h,l,m] = (1/(B*L)) sum_b Ksum[b,h,:].V[b,h,m,:]
    keysum = key.astype(f64).sum(axis=1)                      # [B, D]
    Ksum = (keysum @ Wk.T.astype(f64)).reshape(B, H, DH)      # [B, H, DH]
    WvH = Wv.astype(f64).reshape(H, DH, D)
    wv_eff = np.einsum("hdj,bhd->bhj", WvH, Ksum, optimize=True)      # [B,H,D]
    contrib = np.einsum("bmj,bhj->hm", value.astype(f64), wv_eff, optimize=True)
    mean_kv = contrib / (B * L)                               # [H, DH]
    cs = np.zeros((H, DH), f64)
    c = np.zeros(DH, f64)
    for h in range(H):
        cs[h] = c
        c = b * c + a * mean_kv[h]
    # q_mod = Q*((1-a)*cs + a*kv) = (a*Q)*(kv + (1-a)/a*cs); a is folded
    # into the Q PSUM-copy scale on device, and this is cs*(1-a)/a:
    csp = ((1.0 - a) / a * cs if a != 0 else 0.0 * cs).astype(np.float32)
    csp_dev = np.ascontiguousarray(
        csp.reshape(H, 2, 128).transpose(2, 0, 1).reshape(128, H * 2)
    )
    plain_ln = bool(np.all(ln_g == 1.0) and np.all(ln_b == 0.0))

    bf = ml_dtypes.bfloat16
    f8 = ml_dtypes.float8_e4m3  # TRN fp8e4: max 240, matches bit-for-bit

    def to8(x):
        return np.clip(x, -240.0, 240.0).astype(f8)

    # pack into the exact SBUF tile layouts (one contiguous run per
    # partition per DMA): proj panels [iq*128+p, n*256+c] = wT[n*128+p,
    # iq*256+c]; outproj [(nq*2+jh)*128+p, k*512+c] = woT[(jh*8+k)*128+p,
    # nq*512+c]; activations [p, n*L+l] = xT[n*128+p, l].
    def pack_w(wT):
        arr = np.asarray(wT).reshape(NB, 128, 8, 256)
        return np.ascontiguousarray(
            arr.transpose(2, 1, 0, 3).reshape(8 * 128, NB * 256))

    def pack_wo(woT_):
        arr = np.asarray(woT_).reshape(2, 8, 128, 4, 512)
        return np.ascontiguousarray(
            arr.transpose(3, 0, 2, 1, 4).reshape(8 * 128, 8 * 512))

    def pack_x(xT):
        arr = np.asarray(xT).reshape(NB, 128, L)
        return np.ascontiguousarray(arr.transpose(1, 0, 2).reshape(128, NB * L))

    qT = np.stack([pack_x(to8(query[c].T)) for c in range(B)])
    kT = np.stack([pack_x(to8(key[c].T)) for c in range(B)])
    vT = np.stack([pack_x(to8(value[c].T)) for c in range(B)])
    wqT = pack_w(to8(SW * Wq.T))
    wkT = pack_w(to8(SW * Wk.T))
    wvT = pack_w(to8(SW * Wv.T))
    woT = pack_wo(to8(SW * Wo.T))
    # out-proj PSUM is SW*SA*o; scaling the residual to match makes
    # x_dev = SW*SA*x, and LayerNorm is invariant to uniform scaling.
    # fp16 holds it fine: |SW*SA*x| <= ~2048*6 << 65504.
    qres = (SW * SA * (query + bo[None, None, :])).astype(np.float16)
    # mask[i,l] = 1/SA iff i<=l: folds the fp8 am = A/SA scale into the mask
    mask_full = np.triu(np.full((L, L), 1.0 / SA, np.float32))
    maskT = np.ascontiguousarray(
        mask_full.reshape(LB, 128, L).transpose(1, 0, 2).reshape(128, LB * L)
    ).astype(bf)

    in_maps = []
    for c_ in range(B):
        in_maps.append({
            "qT": qT[c_], "kT": kT[c_], "vT": vT[c_],
            "qres": qres[c_],
            "wqT": wqT, "wkT": wkT, "wvT": wvT, "woT": woT,
            "csp": csp_dev, "maskT": maskT,
            "lng": ln_g.astype(np.float32), "lnb": ln_b.astype(np.float32),
        })
    return in_maps, a, plain_ln


def get_nc(alpha: float, plain_ln: bool = True):
    key = (round(float(alpha), 9), bool(plain_ln))
    if key not in _cache:
        _cache[key] = _build(float(alpha), bool(plain_ln))
    return _cache[key]


def kernel(query, key, value, Wq, Wk, Wv, Wo, bo, ln_g, ln_b, alpha, beta,
           _trace=False, _trace_kwargs=None):
    args = [np.asarray(x) for x in
            (query, key, value, Wq, Wk, Wv, Wo, bo, ln_g, ln_b, alpha, beta)]
    in_maps, a, plain_ln = _host_prep(*args)
    nc = get_nc(a, plain_ln)
    res = run_bass_kernel_spmd(
        nc, in_maps, core_ids=list(range(B)),
        trace=_trace, **(_trace_kwargs or {}),
    )
    out = np.stack([res.results[c]["out"] for c in range(B)], axis=0)
    out = out.astype(np.float32)
    if _trace:
        kernel._last_results = res
    return out



# revision 48
# speedup vs baseline: 1.1967x; 1.0541x over previous
"""DeltaNet attention TRN2 kernel (nn_DeltaNetAttention_5299989643476).

Strategy: data-parallel over batch (8 batches -> 8 NeuronCores). The
cross-batch cumulative_state scan is tiny ([H, Dh]) and is computed on the
host via an algebraic shortcut (mean over (b,l) of kv == Ksum . V
contraction), then passed to every core as a small constant tensor, so the
device program needs no collectives.

On-device, everything runs in a "transposed" layout (features on
partitions, sequence on the free dim):
  - QT/KT/VT projections: weight-stationary fp8e4 DoubleRow matmuls (2x PE
    throughput), fp32 PSUM accum; weights host-prescaled by SW=64 to clear
    the fp8 denormal range, unscaled in the PSUM->SBUF copy
  - per head: kvT matmul; q-mod via tensor_scalar with per-partition cs;
    phi(x)=elu(x)+1 = relu(x)+exp(min(x,0)); causal linear attention as a
    masked A=pq@pk^T matmul; den via an all-ones stationary matmul (which
    also replicates den across partitions for the division broadcast);
    num needs V back in sequence-major layout -> PE transpose
  - output projection + residual + LayerNorm (bn_stats/bn_aggr)
"""

import numpy as np
import ml_dtypes

import concourse.bass as bass
import concourse.mybir as mybir
import concourse.tile as tile
from concourse import bacc
from concourse.bass_utils import run_bass_kernel_spmd
from concourse.masks import make_identity


def _ensure_axon_hooks():
    """This image's `antenv` lacks `axon_hooks`; if the caller's environment
    sets BASS_TRACE, run_bass_kernel_spmd would crash importing it. Register
    a no-op shim (only when absent) so tracing degrades gracefully."""
    try:
        import antenv.axon_hooks  # noqa: F401
    except ImportError:
        import sys
        import types

        import antenv

        mod = types.ModuleType("antenv.axon_hooks")
        _h = [None]
        mod.set_axon_ntff_profile_hook = lambda h: _h.__setitem__(0, h)
        mod.get_axon_ntff_profile_hook = lambda: _h[0]
        sys.modules["antenv.axon_hooks"] = mod
        antenv.axon_hooks = mod


_ensure_axon_hooks()

B, L, D, H = 8, 256, 2048, 8
DH = D // H            # 256
NB = D // 128          # 16 feature blocks of 128
LB = L // 128          # 2 sequence blocks of 128
EPS = 1e-5
SW = 64.0              # fp8 weight pre-scale (power of 2: exact)
SA = 32.0              # fp8 attn pre-scale; SW*SA is folded into qres

F32 = mybir.dt.float32
F16 = mybir.dt.float16
BF16 = mybir.dt.bfloat16
F8 = mybir.dt.float8e4
AF = mybir.ActivationFunctionType
OP = mybir.AluOpType
DR = mybir.MatmulPerfMode.DoubleRow

_cache = {}


def _build(alpha: float, plain_ln: bool = False):
    nc = bacc.Bacc(
        "TRN2",
        target_bir_lowering=False,
        debug=False,
        enable_asserts=False,
        num_devices=B,
    )

    # All big inputs are host-packed into the exact SBUF tile layout so each
    # DMA is 128 partitions x >=2KB contiguous (128 descriptors instead of
    # 2048): the descriptor-generation cost on the issuing queue engine was
    # the round-1 bottleneck (3.4-5.2us per panel).
    qT_d = nc.dram_tensor("qT", [128, NB * L], F8, kind="ExternalInput")
    kT_d = nc.dram_tensor("kT", [128, NB * L], F8, kind="ExternalInput")
    vT_d = nc.dram_tensor("vT", [128, NB * L], F8, kind="ExternalInput")
    qres_d = nc.dram_tensor("qres", [L, D], F16, kind="ExternalInput")
    wqT_d = nc.dram_tensor("wqT", [8 * 128, NB * 256], F8, kind="ExternalInput")
    wkT_d = nc.dram_tensor("wkT", [8 * 128, NB * 256], F8, kind="ExternalInput")
    wvT_d = nc.dram_tensor("wvT", [8 * 128, NB * 256], F8, kind="ExternalInput")
    woT_d = nc.dram_tensor("woT", [8 * 128, 8 * 512], F8, kind="ExternalInput")
    csp_d = nc.dram_tensor("csp", [128, H * 2], F32, kind="ExternalInput")
    maskT_d = nc.dram_tensor("maskT", [128, LB * L], BF16, kind="ExternalInput")
    lng_d = nc.dram_tensor("lng", [D], F32, kind="ExternalInput")
    lnb_d = nc.dram_tensor("lnb", [D], F32, kind="ExternalInput")
    out_d = nc.dram_tensor("out", [L, D], F16, kind="ExternalOutput")

    with tile.TileContext(nc) as tc:
        _body(
            tc, alpha,
            qT_d, kT_d, vT_d, qres_d,
            wqT_d, wkT_d, wvT_d, woT_d,
            csp_d, maskT_d, lng_d, lnb_d, out_d,
            plain_ln,
        )
    nc.compile()
    return nc


def _body(tc, alpha, qT_d, kT_d, vT_d, qres_d, wqT_d, wkT_d, wvT_d, woT_d,
          csp_d, maskT_d, lng_d, lnb_d, out_d, plain_ln):
    nc = tc.nc

    with (
        tc.tile_pool(name="singles", bufs=1) as singles,
        tc.tile_pool(name="wpool", bufs=8) as wpool,
        tc.tile_pool(name="big", bufs=1) as big,
        tc.tile_pool(name="hgrp", bufs=2) as hgrp,
        tc.tile_pool(name="small", bufs=3) as small,
        # one shared 4-deep PSUM pool for projections + kv: their lifetimes
        # are mostly disjoint, so sharing slots doubles each phase's
        # pipelining depth within the same 8-bank budget
        tc.tile_pool(name="psA", bufs=4, space="PSUM") as psA,
        tc.tile_pool(name="an_ps", bufs=2, space="PSUM") as an_ps,
        tc.tile_pool(name="dv_ps", bufs=2, space="PSUM") as dv_ps,
    ):
        # ---- projections: XT[i, l] = sum_j WT[j, i] * xT[j, l] ----
        # K first (pk depends only on K), then V (kv + transposes), then Q.
        # Inputs stream on the gpsimd queue, weights on the sync queue, so
        # their issue costs overlap. The K input DMA goes first on gpsimd.
        xT_in = {}
        for name, dram in (("k", kT_d), ("v", vT_d), ("q", qT_d)):
            t = big.tile([128, NB, L], F8, tag=f"{name}T_in", name=f"{name}T_in")
            xT_in[name] = (t, dram)

        def load_xT(name):
            t, dram = xT_in[name]
            tf = t.rearrange("p n l -> p (n l)")
            # halves so the first j-blocks unblock matmuls sooner. K loads at
            # t=0 when DVE is idle, so gpsimd's SWDGE is safe; the V/Q loads
            # happen while DVE runs fp32 PSUM copies, which lock the shared
            # SBUF port and starve SWDGE - route those via HWDGE (scalar).
            eng1 = nc.gpsimd if name == "k" else nc.scalar
            eng2 = nc.sync if name == "k" else nc.scalar
            eng1.dma_start(out=tf[:, 0:8 * L], in_=dram.ap()[:, 0:8 * L])
            eng2.dma_start(out=tf[:, 8 * L:NB * L], in_=dram.ap()[:, 8 * L:NB * L])

        load_xT("k")

        # constants after the K input on the gpsimd queue
        ident8 = singles.tile([128, 128], F8)
        make_identity(nc, ident8)
        # fp8 ones at 1/SA: den_ps = rowsum(am)/SA^2 with am = A/SA, so
        # rden = SA^2/den and attnT = num_ps*rden = SA*attn
        ones_t = singles.tile([128, 2, 128], F8)
        nc.vector.memset(ones_t, 1.0 / SA)
        eps_t = singles.tile([128, 1], F32)
        nc.vector.memset(eps_t, EPS)
        csp_t = singles.tile([128, H * 2], F32)
        nc.gpsimd.dma_start(out=csp_t, in_=csp_d.ap())

        # dummy matmuls while the first weight panels stream in: keeps the
        # PE-HAM activity monitor busy so the real stream starts at 2.4 GHz
        warm_ps = dv_ps.tile([128, 256], F32, tag="dv", name="warm_ps")
        for _ in range(24):
            nc.tensor.matmul(warm_ps[:, 0:128], ones_t[:, 0, :], ones_t[:, 0, :],
                             start=True, stop=True)

        w_ds = {"k": wkT_d, "v": wvT_d, "q": wqT_d, "o": woT_d}
        succ = {"k": "v", "v": "q", "q": "o"}
        prefetched = {}

        def panel_dma(name, iq, tag, halved=False):
            w_t = wpool.tile([128, NB, 256], F8, tag=tag, name=f"w_{name}{iq}")
            wf = w_t.rearrange("p n i -> p (n i)")
            rows = slice(iq * 128, (iq + 1) * 128)
            w_r = w_ds[name].ap()[rows, :]
            # alternate issue queues during the projections (ScalarE is idle
            # there) so issue latency and transfers overlap; outproj panels
            # stay on sync (ScalarE has real work by then)
            eng = nc.scalar if (name != "o" and iq % 2 == 1) else nc.sync
            if halved:
                eng.dma_start(out=wf[:, 0:8 * 256], in_=w_r[:, 0:8 * 256])
                eng.dma_start(out=wf[:, 8 * 256:NB * 256], in_=w_r[:, 8 * 256:NB * 256])
            else:
                eng.dma_start(out=wf, in_=w_r)
            return w_t

        # PSUM holds SW*X (fp8 weights are pre-scaled by SW on the host);
        # the copy to SBUF unscales — and folds alpha for Q. K/V land in fp8
        # (they only feed fp8 matmuls + phi); Q stays bf16 for the q-mod mul.
        unscale = {"k": 1.0 / SW, "v": 1.0 / SW, "q": alpha / SW}
        proj_dt = {"k": F8, "v": F8, "q": BF16}
        projs = {
            name: big.tile([128, NB, L], proj_dt[name], tag=f"{name}proj",
                           name=f"{name}proj")
            for name in ("k", "v", "q")
        }
        KT_t, VT_t, QT_t = projs["k"], projs["v"], projs["q"]

        def proj_iq(name, iq):
            w_t = panel_dma(name, iq, "w", halved=(name == "k" and iq < 4))
            if iq == 3 and succ[name] != "o":
                # next projection's activation streams during this proj
                load_xT(succ[name])
            out_t, x_t = projs[name], xT_in[name][0]
            ps = psA.tile([128, 2, L], F32, tag="pk")
            for ib in range(2):
                for j in range(0, NB, 2):  # DoubleRow: 2 k-blocks/matmul
                    nc.tensor.matmul(
                        ps[:, ib, :],
                        w_t[:, j:j + 2, ib * 128:(ib + 1) * 128],
                        x_t[:, j:j + 2, :],
                        start=(j == 0),
                        stop=(j == NB - 2),
                        perf_mode=DR,
                    )
            nc.vector.tensor_scalar(
                out=out_t[:, iq * 2:iq * 2 + 2, :], in0=ps,
                scalar1=unscale[name], scalar2=None, op0=OP.mult,
            )

        # K and V run to completion; Q is interleaved with the attention
        # head groups below (group g only needs Q i-quarters 2g, 2g+1), so
        # the attention phase's DVE/ACT chains hide under Q-proj matmuls.
        for name in ("k", "v"):
            for iq in range(8):
                proj_iq(name, iq)

        maskT_t = singles.tile([128, LB, L], BF16)
        nc.gpsimd.dma_start(out=maskT_t.rearrange("p a l -> p (a l)"),
                            in_=maskT_d.ap())
        qres_t = []
        for lb in range(LB):
            t = big.tile([128, D], F16, tag=f"qres{lb}", name=f"qres{lb}")
            nc.scalar.dma_start(out=t, in_=qres_d.ap()[lb * 128:(lb + 1) * 128, :])
            qres_t.append(t)
        lng_t = lnb_t = None
        if not plain_ln:
            lng_t = singles.tile([128, D], F32)
            nc.gpsimd.dma_start(out=lng_t,
                                in_=lng_d.ap().partition_broadcast(128))
            lnb_t = singles.tile([128, D], F32)
            nc.gpsimd.dma_start(out=lnb_t,
                                in_=lnb_d.ap().partition_broadcast(128))

        # ---- pk = phi(KT) over all heads at once (fp8 out) ----
        # phi(x) = max(x,0) + exp(min(x,0)) in 2 DVE ops + 1 ACT op: the
        # max+add collapse into one STT.
        pk_t = big.tile([128, NB, L], F8, tag="pk")
        ek_t = big.tile([128, NB, L], BF16, tag="ek")
        nc.vector.tensor_scalar_min(ek_t, KT_t, 0.0)
        nc.scalar.activation(ek_t, ek_t, AF.Exp)
        nc.vector.scalar_tensor_tensor(
            out=pk_t, in0=KT_t, scalar=0.0, in1=ek_t,
            op0=OP.max, op1=OP.add,
        )

        # ---- per-head-group (2 heads): kv + V-transpose + q-mod + phi(q) ----
        # V-transposes ride along per group so PE has filler work while the
        # group's phi chain runs on DVE/ACT.
        V_t = big.tile([128, LB, D], F8, tag="V")
        pq_t = big.tile([128, NB, L], F8, tag="pq")

        def group_block(g):
            kvm = hgrp.tile([128, 4, L], BF16, tag="kvm")
            kva = hgrp.tile([128, 4, L], BF16, tag="kva")
            for hh in range(2):
                h = 2 * g + hh
                n0 = 2 * h
                ps = psA.tile([128, 2, L], F32, tag="pk")
                for mb in range(2):
                    nc.tensor.matmul(
                        ps[:, mb, :],
                        VT_t[:, n0:n0 + 2, mb * 128:(mb + 1) * 128],
                        KT_t[:, n0:n0 + 2, :],
                        start=True, stop=True,
                        perf_mode=DR,
                    )
                for mb in range(2):
                    # q_mod = (alpha*Q) * (kv + cs*(1-alpha)/alpha), with
                    # alpha folded into the Q unscale. The +cs is a per-
                    # partition bias -> ScalarE; the multiply runs on DVE at
                    # the cheap bf16 SBUF rate instead of one slow PSUM STT.
                    nc.scalar.activation(
                        out=kva[:, 2 * hh + mb, :], in_=ps[:, mb, :],
                        func=AF.Identity,
                        bias=csp_t[:, n0 + mb:n0 + mb + 1],
                    )
                    nc.vector.tensor_mul(
                        kvm[:, 2 * hh + mb, :], kva[:, 2 * hh + mb, :],
                        QT_t[:, n0 + mb, :],
                    )
                for ib in range(LB):
                    # fp8 PE transpose writes 16-bit granules: the output AP
                    # must step by 2 elements; the copy below compacts it.
                    psv = dv_ps.tile([128, 512], F8, tag="dv")
                    for db in range(2):
                        nc.tensor.transpose(
                            psv[:, db * 256:(db + 1) * 256:2],
                            VT_t[:, n0 + db, ib * 128:(ib + 1) * 128],
                            ident8,
                        )
                    nc.scalar.copy(
                        out=V_t[:, ib, h * DH:h * DH + 256],
                        in_=psv[:, 0:512:2],
                    )
            # pq = phi(q_mod): min -> Exp -> fused max+add STT
            qsl = slice(4 * g, 4 * g + 4)
            eq = hgrp.tile([128, 4, L], BF16, tag="eq")
            nc.vector.tensor_scalar_min(eq, kvm, 0.0)
            nc.scalar.activation(eq, eq, AF.Exp)
            nc.vector.scalar_tensor_tensor(
                out=pq_t[:, qsl, :], in0=kvm, scalar=0.0, in1=eq,
                op0=OP.max, op1=OP.add,
            )

        # ---- per-head: A matmul, mask, den, num, outT ----
        # attnT holds SA*attn in fp8 (|SA*attn| <= ~150 < 240 max)
        attnT_t = big.tile([128, NB, L], F8, tag="attnT")

        def head_block(h):
            n0 = 2 * h
            # causal block structure of AT[i, l] (i<=l kept):
            #   ib=0: l<128 lower-triangular, l>=128 all-ones
            #   ib=1: l<128 all-zero (skipped entirely), l>=128 triangular
            a_ps = an_ps.tile([128, 2, L], F32, tag="an")
            nc.tensor.matmul(
                a_ps[:, 0, :],
                pk_t[:, n0:n0 + 2, 0:128],
                pq_t[:, n0:n0 + 2, :],
                start=True, stop=True, perf_mode=DR,
            )
            nc.tensor.matmul(
                a_ps[:, 1, 128:L],
                pk_t[:, n0:n0 + 2, 128:L],
                pq_t[:, n0:n0 + 2, 128:L],
                start=True, stop=True, perf_mode=DR,
            )
            # am = A/SA in fp8 (maskT holds 1/SA); the always-unmasked
            # middle block gets the 1/SA scale on ScalarE
            am = small.tile([128, LB, L], F8, tag="am")
            nc.vector.tensor_mul(am[:, 0, 0:128], a_ps[:, 0, 0:128],
                                 maskT_t[:, 0, 0:128])
            nc.scalar.activation(out=am[:, 0, 128:L], in_=a_ps[:, 0, 128:L],
                                 func=AF.Identity, scale=1.0 / SA)
            nc.vector.tensor_mul(am[:, 1, 128:L], a_ps[:, 1, 128:L],
                                 maskT_t[:, 1, 128:L])

            den_ps = dv_ps.tile([128, L], F32, tag="dv", name="den_ps")
            nc.tensor.matmul(den_ps[:, 0:128], ones_t[:, 0, :], am[:, 0, 0:128],
                             start=True, stop=True)
            nc.tensor.matmul(den_ps[:, 128:L], ones_t, am[:, 0:2, 128:L],
                             start=True, stop=True, perf_mode=DR)
            # den is a sum of strictly positive phi-products (>= O(0.01)
            # mathematically, O(100) in practice), so the reference's 1e-8
            # clamp can never bind — reciprocal straight from PSUM.
            rden = small.tile([128, L], F32, tag="rden")
            nc.vector.reciprocal_approx_fast(out=rden, in_=den_ps)

            n_ps = an_ps.tile([128, 2, L], F32, tag="an")
            for db in range(2):
                dsl = slice(h * DH + db * 128, h * DH + (db + 1) * 128)
                nc.tensor.matmul(n_ps[:, db, 0:128], V_t[:, 0, dsl],
                                 am[:, 0, 0:128], start=True, stop=True)
                nc.tensor.matmul(n_ps[:, db, 128:L], V_t[:, 0:2, dsl],
                                 am[:, 0:2, 128:L],
                                 start=True, stop=True, perf_mode=DR)
            for db in range(2):
                nc.vector.tensor_mul(attnT_t[:, n0 + db, :], n_ps[:, db, :], rden)

        # ---- interleaved Q projection + attention pipeline ----
        # group g's whole chain only needs Q i-quarters 2g/2g+1, so its
        # DVE/ACT work hides under the next group's Q-proj matmuls.
        for g in range(4):
            proj_iq("q", 2 * g)
            proj_iq("q", 2 * g + 1)
            group_block(g)
            head_block(2 * g)
            head_block(2 * g + 1)

        # Trigger the sqrt ACT-table load now — after ScalarE's last
        # Copy/Exp user, off the LN tail's critical path (the set switch
        # costs ~2.6us).
        warm_sqrt = singles.tile([128, 1], F32)
        nc.scalar.activation(warm_sqrt, eps_t, AF.Sqrt)

        # ---- output projection + residual + LayerNorm ----
        # 512-wide moving operand: half the matmul and LDWEIGHTS count of
        # the input projections. All 8 panels are preloaded (wpool bufs=8)
        # so the loop can run lb-major: block 0's LayerNorm overlaps block
        # 1's matmuls instead of serializing at the tail.
        x_sb = [big.tile([128, D], F32, tag=f"x{lb}", name=f"x{lb}")
                for lb in range(LB)]
        stats = [small.tile([128, 4, 6], F32, tag=f"stats{lb}",
                            name=f"stats{lb}", bufs=1) for lb in range(LB)]
        wo_tiles = []
        for nq in range(4):
            wo = []
            for jh in range(2):
                w_t = wpool.tile([128, 8, 512], F8, tag="w",
                                 name=f"w_o{nq}{jh}")
                rows = slice((nq * 2 + jh) * 128, (nq * 2 + jh + 1) * 128)
                nc.sync.dma_start(
                    out=w_t.rearrange("p n i -> p (n i)"),
                    in_=w_ds["o"].ap()[rows, :])
                wo.append(w_t)
            wo_tiles.append(wo)

        def outproj_block(lb):
            for nq in range(4):
                wo = wo_tiles[nq]
                ps = psA.tile([128, 2, L], F32, tag="pk")
                psf = ps.rearrange("p a l -> p (a l)")
                for j in range(0, NB, 2):  # DoubleRow k-block pairs
                    nc.tensor.matmul(
                        psf,
                        attnT_t[:, j:j + 2, lb * 128:(lb + 1) * 128],
                        wo[j // 8][:, j % 8:j % 8 + 2, :],
                        start=(j == 0),
                        stop=(j == NB - 2),
                        perf_mode=DR,
                    )
                # x = o + (query + bo)
                sl = slice(nq * 512, (nq + 1) * 512)
                nc.vector.tensor_add(x_sb[lb][:, sl], psf, qres_t[lb][:, sl])
                # LN stats pipelined per 512-chunk while later chunks project
                nc.vector.bn_stats(out=stats[lb][:, nq, :],
                                   in_=x_sb[lb][:, sl])

        def ln_block(lb):
            x = x_sb[lb]
            x16 = big.tile([128, D], F16, tag=f"x16{lb}", name=f"x16{lb}")
            mv = small.tile([128, 2], F32, tag="mv")
            nc.vector.bn_aggr(out=mv, in_=stats[lb])
            sd = small.tile([128, 1], F32, tag="sd")
            nc.scalar.activation(sd, mv[:, 1:2], AF.Sqrt, bias=eps_t)
            nc.vector.reciprocal_approx_fast(out=sd, in_=sd)
            for ch in range(4):  # quarters, so DVE work overlaps output DMA
                sl = slice(ch * (D // 4), (ch + 1) * (D // 4))
                if plain_ln:
                    # ln_g == 1, ln_b == 0: fused (x - mu) * rstd. All on
                    # DVE: a ScalarE Identity here would force Sqrt<->
                    # Identity ACT-table swaps (~1.3us each) in the tail.
                    nc.vector.tensor_scalar(
                        out=x16[:, sl], in0=x[:, sl], scalar1=mv[:, 0:1],
                        scalar2=sd, op0=OP.subtract, op1=OP.mult,
                    )
                else:
                    nc.vector.tensor_scalar(
                        out=x[:, sl], in0=x[:, sl], scalar1=mv[:, 0:1],
                        scalar2=None, op0=OP.subtract,
                    )
                    nc.vector.scalar_tensor_tensor(
                        out=x[:, sl], in0=x[:, sl], scalar=sd, in1=lng_t[:, sl],
                        op0=OP.mult, op1=OP.mult,
                    )
                    nc.vector.tensor_add(x16[:, sl], x[:, sl], lnb_t[:, sl])
                # alternate output-DMA issue queues (both HWDGE: gpsimd's
                # SWDGE would starve behind the DVE LN ops here)
                oeng = nc.sync if ch % 2 == 0 else nc.scalar
                oeng.dma_start(
                    out=out_d.ap()[lb * 128:(lb + 1) * 128, sl], in_=x16[:, sl])

        for lb in range(LB):
            outproj_block(lb)
            ln_block(lb)


def _host_prep(query, key, value, Wq, Wk, Wv, Wo, bo, ln_g, ln_b, alpha, beta):
    """Host-side: cumulative_state shortcut + layout/dtype marshaling."""
    a, b = float(alpha), float(beta)
    f64 = np.float64
    # mean over (batch, l) of kv[b,h,l,m] = (1/(B*L)) sum_b Ksum[b,h,:].V[b,h,m,:]
    keysum = key.astype(f64).sum(axis=1)                      # [B, D]
    Ksum = (keysum @ Wk.T.astype(f64)).reshape(B, H, DH)      # [B, H, DH]
    WvH = Wv.astype(f64).reshape(H, DH, D)
    wv_eff = np.einsum("hdj,bhd->bhj", WvH, Ksum, optimize=True)      # [B,H,D]
    contrib = np.einsum("bmj,bhj->hm", value.astype(f64), wv_eff, optimize=True)
    mean_kv = contrib / (B * L)                               # [H, DH]
    cs = np.zeros((H, DH), f64)
    c = np.zeros(DH, f64)
    for h in range(H):
        cs[h] = c
        c = b * c + a * mean_kv[h]
    # q_mod = Q*((1-a)*cs + a*kv) = (a*Q)*(kv + (1-a)/a*cs); a is folded
    # into the Q PSUM-copy scale on device, and this is cs*(1-a)/a:
    csp = ((1.0 - a) / a * cs if a != 0 else 0.0 * cs).astype(np.float32)
    csp_dev = np.ascontiguousarray(
        csp.reshape(H, 2, 128).transpose(2, 0, 1).reshape(128, H * 2)
    )
    plain_ln = bool(np.all(ln_g == 1.0) and np.all(ln_b == 0.0))

    bf = ml_dtypes.bfloat16
    f8 = ml_dtypes.float8_e4m3  # TRN fp8e4: max 240, matches bit-for-bit

    def to8(x):
        return np.clip(x, -240.0, 240.0).astype(f8)

    # pack into the exact SBUF tile layouts (one contiguous run per
    # partition per DMA): proj panels [iq*128+p, n*256+c] = wT[n*128+p,
    # iq*256+c]; outproj [(nq*2+jh)*128+p, k*512+c] = woT[(jh*8+k)*128+p,
    # nq*512+c]; activations [p, n*L+l] = xT[n*128+p, l].
    def pack_w(wT):
        arr = np.asarray(wT).reshape(NB, 128, 8, 256)
        return np.ascontiguousarray(
            arr.transpose(2, 1, 0, 3).reshape(8 * 128, NB * 256))

    def pack_wo(woT_):
        arr = np.asarray(woT_).reshape(2, 8, 128, 4, 512)
        return np.ascontiguousarray(
            arr.transpose(3, 0, 2, 1, 4).reshape(8 * 128, 8 * 512))

    def pack_x(xT):
        arr = np.asarray(xT).reshape(NB, 128, L)
        return np.ascontiguousarray(arr.transpose(1, 0, 2).reshape(128, NB * L))

    qT = np.stack([pack_x(to8(query[c].T)) for c in range(B)])
    kT = np.stack([pack_x(to8(key[c].T)) for c in range(B)])
    vT = np.stack([pack_x(to8(value[c].T)) for c in range(B)])
    wqT = pack_w(to8(SW * Wq.T))
    wkT = pack_w(to8(SW * Wk.T))
    wvT = pack_w(to8(SW * Wv.T))
    woT = pack_wo(to8(SW * Wo.T))
    # out-proj PSUM is SW*SA*o; scaling the residual to match makes
    # x_dev = SW*SA*x, and LayerNorm is invariant to uniform scaling.
    # fp16 holds it fine: |SW*SA*x| <= ~2048*6 << 65504.
    qres = (SW * SA * (query + bo[None, None, :])).astype(np.float16)
    # mask[i,l] = 1/SA iff i<=l: folds the fp8 am = A/SA scale into the mask
    mask_full = np.triu(np.full((L, L), 1.0 / SA, np.float32))
    maskT = np.ascontiguousarray(
        mask_full.reshape(LB, 128, L).transpose(1, 0, 2).reshape(128, LB * L)
    ).astype(bf)

    in_maps = []
    for c_ in range(B):
        in_maps.append({
            "qT": qT[c_], "kT": kT[c_], "vT": vT[c_],
            "qres": qres[c_],
            "wqT": wqT, "wkT": wkT, "wvT": wvT, "woT": woT,
            "csp": csp_dev, "maskT": maskT,
            "lng": ln_g.astype(np.float32), "lnb": ln_b.astype(np.float32),
        })
    return in_maps, a, plain_ln


def get_nc(alpha: float, plain_ln: bool = True):
    key = (round(float(alpha), 9), bool(plain_ln))
    if key not in _cache:
        _cache[key] = _build(float(alpha), bool(plain_ln))
    return _cache[key]


def kernel(query, key, value, Wq, Wk, Wv, Wo, bo, ln_g, ln_b, alpha, beta,
           _trace=False, _trace_kwargs=None):
    args = [np.asarray(x) for x in
            (query, key, value, Wq, Wk, Wv, Wo, bo, ln_g, ln_b, alpha, beta)]
    in_maps, a, plain_ln = _host_prep(*args)
    nc = get_nc(a, plain_ln)
    res = run_bass_kernel_spmd(
        nc, in_maps, core_ids=list(range(B)),
        trace=_trace, **(_trace_kwargs or {}),
    )
    out = np.stack([res.results[c]["out"] for c in range(B)], axis=0)
    out = out.astype(np.float32)
    if _trace:
        kernel._last_results = res
    return out



# revision 51
# speedup vs baseline: 1.2355x; 1.0324x over previous
"""DeltaNet attention TRN2 kernel (nn_DeltaNetAttention_5299989643476).

Strategy: data-parallel over batch (8 batches -> 8 NeuronCores). The
cross-batch cumulative_state scan is tiny ([H, Dh]) and is computed on the
host via an algebraic shortcut (mean over (b,l) of kv == Ksum . V
contraction), then passed to every core as a small constant tensor, so the
device program needs no collectives.

On-device, everything runs in a "transposed" layout (features on
partitions, sequence on the free dim):
  - QT/KT/VT projections: weight-stationary fp8e4 DoubleRow matmuls (2x PE
    throughput), fp32 PSUM accum; weights host-prescaled by SW=64 to clear
    the fp8 denormal range, unscaled in the PSUM->SBUF copy
  - per head: kvT matmul; q-mod via tensor_scalar with per-partition cs;
    phi(x)=elu(x)+1 = relu(x)+exp(min(x,0)); causal linear attention as a
    masked A=pq@pk^T matmul; den via an all-ones stationary matmul (which
    also replicates den across partitions for the division broadcast);
    num needs V back in sequence-major layout -> PE transpose
  - output projection + residual + LayerNorm (bn_stats/bn_aggr)
"""

import numpy as np
import ml_dtypes

import concourse.bass as bass
import concourse.mybir as mybir
import concourse.tile as tile
from concourse import bacc
from concourse.bass_utils import run_bass_kernel_spmd
from concourse.masks import make_identity


def _ensure_axon_hooks():
    """This image's `antenv` lacks `axon_hooks`; if the caller's environment
    sets BASS_TRACE, run_bass_kernel_spmd would crash importing it. Register
    a no-op shim (only when absent) so tracing degrades gracefully."""
    try:
        import antenv.axon_hooks  # noqa: F401
    except ImportError:
        import sys
        import types

        import antenv

        mod = types.ModuleType("antenv.axon_hooks")
        _h = [None]
        mod.set_axon_ntff_profile_hook = lambda h: _h.__setitem__(0, h)
        mod.get_axon_ntff_profile_hook = lambda: _h[0]
        sys.modules["antenv.axon_hooks"] = mod
        antenv.axon_hooks = mod


_ensure_axon_hooks()

B, L, D, H = 8, 256, 2048, 8
DH = D // H            # 256
NB = D // 128          # 16 feature blocks of 128
LB = L // 128          # 2 sequence blocks of 128
EPS = 1e-5
SW = 64.0              # fp8 weight pre-scale (power of 2: exact)
SA = 32.0              # fp8 attn pre-scale; SW*SA is folded into qres

F32 = mybir.dt.float32
F16 = mybir.dt.float16
BF16 = mybir.dt.bfloat16
F8 = mybir.dt.float8e4
AF = mybir.ActivationFunctionType
OP = mybir.AluOpType
DR = mybir.MatmulPerfMode.DoubleRow

_cache = {}


def _build(alpha: float, plain_ln: bool = False):
    nc = bacc.Bacc(
        "TRN2",
        target_bir_lowering=False,
        debug=False,
        enable_asserts=False,
        num_devices=B,
    )

    # All big inputs are host-packed into the exact SBUF tile layout so each
    # DMA is 128 partitions x >=2KB contiguous (128 descriptors instead of
    # 2048): the descriptor-generation cost on the issuing queue engine was
    # the round-1 bottleneck (3.4-5.2us per panel).
    qT_d = nc.dram_tensor("qT", [128, NB * L], F8, kind="ExternalInput")
    kT_d = nc.dram_tensor("kT", [128, NB * L], F8, kind="ExternalInput")
    vT_d = nc.dram_tensor("vT", [128, NB * L], F8, kind="ExternalInput")
    qres_d = nc.dram_tensor("qres", [L, D], F16, kind="ExternalInput")
    wqT_d = nc.dram_tensor("wqT", [8 * 128, NB * 256], F8, kind="ExternalInput")
    wkT_d = nc.dram_tensor("wkT", [8 * 128, NB * 256], F8, kind="ExternalInput")
    wvT_d = nc.dram_tensor("wvT", [8 * 128, NB * 256], F8, kind="ExternalInput")
    woT_d = nc.dram_tensor("woT", [8 * 128, 8 * 512], F8, kind="ExternalInput")
    csp_d = nc.dram_tensor("csp", [128, H * 2], F32, kind="ExternalInput")
    maskT_d = nc.dram_tensor("maskT", [128, LB * L], BF16, kind="ExternalInput")
    lng_d = nc.dram_tensor("lng", [D], F32, kind="ExternalInput")
    lnb_d = nc.dram_tensor("lnb", [D], F32, kind="ExternalInput")
    out_d = nc.dram_tensor("out", [L, D], F16, kind="ExternalOutput")

    with tile.TileContext(nc) as tc:
        _body(
            tc, alpha,
            qT_d, kT_d, vT_d, qres_d,
            wqT_d, wkT_d, wvT_d, woT_d,
            csp_d, maskT_d, lng_d, lnb_d, out_d,
            plain_ln,
        )
    nc.compile()
    return nc


def _body(tc, alpha, qT_d, kT_d, vT_d, qres_d, wqT_d, wkT_d, wvT_d, woT_d,
          csp_d, maskT_d, lng_d, lnb_d, out_d, plain_ln):
    nc = tc.nc

    with (
        tc.tile_pool(name="singles", bufs=1) as singles,
        tc.tile_pool(name="wpool", bufs=8) as wpool,
        tc.tile_pool(name="big", bufs=1) as big,
        tc.tile_pool(name="hgrp", bufs=3) as hgrp,
        tc.tile_pool(name="small", bufs=4) as small,
        # one shared 4-deep PSUM pool for projections + kv: their lifetimes
        # are mostly disjoint, so sharing slots doubles each phase's
        # pipelining depth within the same 8-bank budget
        tc.tile_pool(name="psA", bufs=4, space="PSUM") as psA,
        tc.tile_pool(name="an_ps", bufs=2, space="PSUM") as an_ps,
        tc.tile_pool(name="dv_ps", bufs=2, space="PSUM") as dv_ps,
    ):
        # ---- projections: XT[i, l] = sum_j WT[j, i] * xT[j, l] ----
        # K first (pk depends only on K), then V (kv + transposes), then Q.
        # Inputs stream on the gpsimd queue, weights on the sync queue, so
        # their issue costs overlap. The K input DMA goes first on gpsimd.
        xT_in = {}
        for name, dram in (("k", kT_d), ("v", vT_d), ("q", qT_d)):
            t = big.tile([128, NB, L], F8, tag=f"{name}T_in", name=f"{name}T_in")
            xT_in[name] = (t, dram)

        def load_xT(name):
            t, dram = xT_in[name]
            tf = t.rearrange("p n l -> p (n l)")
            # halves so the first j-blocks unblock matmuls sooner. K loads at
            # t=0 when DVE is idle, so gpsimd's SWDGE is safe; the V/Q loads
            # happen while DVE runs fp32 PSUM copies, which lock the shared
            # SBUF port and starve SWDGE - route those via HWDGE (scalar).
            eng1 = nc.gpsimd if name == "k" else nc.scalar
            eng2 = nc.sync if name == "k" else nc.scalar
            eng1.dma_start(out=tf[:, 0:8 * L], in_=dram.ap()[:, 0:8 * L])
            eng2.dma_start(out=tf[:, 8 * L:NB * L], in_=dram.ap()[:, 8 * L:NB * L])

        load_xT("k")

        # constants after the K input on the gpsimd queue
        ident8 = singles.tile([128, 128], F8)
        make_identity(nc, ident8)
        # fp8 ones at 1/SA: den_ps = rowsum(am)/SA^2 with am = A/SA, so
        # rden = SA^2/den and attnT = num_ps*rden = SA*attn
        ones_t = singles.tile([128, 2, 128], F8)
        nc.vector.memset(ones_t, 1.0 / SA)
        eps_t = singles.tile([128, 1], F32)
        nc.vector.memset(eps_t, EPS)
        csp_t = singles.tile([128, H * 2], F32)
        nc.gpsimd.dma_start(out=csp_t, in_=csp_d.ap())

        # dummy matmuls while the first weight panels stream in: keeps the
        # PE-HAM activity monitor busy so the real stream starts at 2.4 GHz
        warm_ps = dv_ps.tile([128, 256], F32, tag="dv", name="warm_ps")
        for _ in range(24):
            nc.tensor.matmul(warm_ps[:, 0:128], ones_t[:, 0, :], ones_t[:, 0, :],
                             start=True, stop=True)

        w_ds = {"k": wkT_d, "v": wvT_d, "q": wqT_d, "o": woT_d}
        succ = {"k": "v", "v": "q", "q": "o"}
        prefetched = {}

        def panel_dma(name, iq, tag, halved=False):
            w_t = wpool.tile([128, NB, 256], F8, tag=tag, name=f"w_{name}{iq}")
            wf = w_t.rearrange("p n i -> p (n i)")
            rows = slice(iq * 128, (iq + 1) * 128)
            w_r = w_ds[name].ap()[rows, :]
            # K panels round-robin 3 queues: each DMA ring sustains only a
            # fraction of HBM bw, and the start of the kernel is bound by
            # how fast the first panels land. gpsimd (SWDGE) is safe for K
            # only - at t~0 DVE is idle so descriptor-gen can't starve.
            if name == "k":
                eng = (nc.sync, nc.scalar, nc.gpsimd)[iq % 3]
            else:
                eng = nc.scalar if (name != "o" and iq % 2 == 1) else nc.sync
            if halved:
                eng.dma_start(out=wf[:, 0:8 * 256], in_=w_r[:, 0:8 * 256])
                eng.dma_start(out=wf[:, 8 * 256:NB * 256], in_=w_r[:, 8 * 256:NB * 256])
            else:
                eng.dma_start(out=wf, in_=w_r)
            return w_t

        # PSUM holds SW*X (fp8 weights are pre-scaled by SW on the host);
        # the copy to SBUF unscales — and folds alpha for Q. K/V land in fp8
        # (they only feed fp8 matmuls + phi); Q stays bf16 for the q-mod mul.
        unscale = {"k": 1.0 / SW, "v": 1.0 / SW, "q": alpha / SW}
        proj_dt = {"k": F8, "v": F8, "q": BF16}
        projs = {
            name: big.tile([128, NB, L], proj_dt[name], tag=f"{name}proj",
                           name=f"{name}proj")
            for name in ("k", "v", "q")
        }
        KT_t, VT_t, QT_t = projs["k"], projs["v"], projs["q"]

        def proj_iq(name, iq):
            w_t = panel_dma(name, iq, "w", halved=(name == "k" and iq < 4))
            if iq == 3 and succ[name] != "o":
                # next projection's activation streams during this proj
                load_xT(succ[name])
            out_t, x_t = projs[name], xT_in[name][0]
            ps = psA.tile([128, 2, L], F32, tag="pk")
            for ib in range(2):
                for j in range(0, NB, 2):  # DoubleRow: 2 k-blocks/matmul
                    nc.tensor.matmul(
                        ps[:, ib, :],
                        w_t[:, j:j + 2, ib * 128:(ib + 1) * 128],
                        x_t[:, j:j + 2, :],
                        start=(j == 0),
                        stop=(j == NB - 2),
                        perf_mode=DR,
                    )
            nc.vector.tensor_scalar(
                out=out_t[:, iq * 2:iq * 2 + 2, :], in0=ps,
                scalar1=unscale[name], scalar2=None, op0=OP.mult,
            )

        # K and V run to completion; Q is interleaved with the attention
        # head groups below (group g only needs Q i-quarters 2g, 2g+1), so
        # the attention phase's DVE/ACT chains hide under Q-proj matmuls.
        for name in ("k", "v"):
            for iq in range(8):
                proj_iq(name, iq)

        maskT_t = singles.tile([128, LB, L], BF16)
        nc.gpsimd.dma_start(out=maskT_t.rearrange("p a l -> p (a l)"),
                            in_=maskT_d.ap())
        qres_t = []
        for lb in range(LB):
            t = big.tile([128, D], F16, tag=f"qres{lb}", name=f"qres{lb}")
            nc.scalar.dma_start(out=t, in_=qres_d.ap()[lb * 128:(lb + 1) * 128, :])
            qres_t.append(t)
        lng_t = lnb_t = None
        if not plain_ln:
            lng_t = singles.tile([128, D], F32)
            nc.gpsimd.dma_start(out=lng_t,
                                in_=lng_d.ap().partition_broadcast(128))
            lnb_t = singles.tile([128, D], F32)
            nc.gpsimd.dma_start(out=lnb_t,
                                in_=lnb_d.ap().partition_broadcast(128))

        # ---- pk = phi(KT) over all heads at once (fp8 out) ----
        # phi(x) = max(x,0) + exp(min(x,0)) in 2 DVE ops + 1 ACT op: the
        # max+add collapse into one STT.
        pk_t = big.tile([128, NB, L], F8, tag="pk")
        ek_t = big.tile([128, NB, L], BF16, tag="ek")
        nc.vector.tensor_scalar_min(ek_t, KT_t, 0.0)
        nc.scalar.activation(ek_t, ek_t, AF.Exp)
        nc.vector.scalar_tensor_tensor(
            out=pk_t, in0=KT_t, scalar=0.0, in1=ek_t,
            op0=OP.max, op1=OP.add,
        )

        # ---- per-head-group (2 heads): kv + V-transpose + q-mod + phi(q) ----
        # V-transposes ride along per group so PE has filler work while the
        # group's phi chain runs on DVE/ACT.
        V_t = big.tile([128, LB, D], F8, tag="V")
        pq_t = big.tile([128, NB, L], F8, tag="pq")

        def group_block(g):
            kvm = hgrp.tile([128, 4, L], BF16, tag="kvm")
            kva = hgrp.tile([128, 4, L], BF16, tag="kva")
            for hh in range(2):
                h = 2 * g + hh
                n0 = 2 * h
                ps = psA.tile([128, 2, L], F32, tag="pk")
                for mb in range(2):
                    nc.tensor.matmul(
                        ps[:, mb, :],
                        VT_t[:, n0:n0 + 2, mb * 128:(mb + 1) * 128],
                        KT_t[:, n0:n0 + 2, :],
                        start=True, stop=True,
                        perf_mode=DR,
                    )
                for mb in range(2):
                    # q_mod = (alpha*Q) * (kv + cs*(1-alpha)/alpha), with
                    # alpha folded into the Q unscale. The +cs is a per-
                    # partition bias -> ScalarE; the multiply runs on DVE at
                    # the cheap bf16 SBUF rate instead of one slow PSUM STT.
                    nc.scalar.activation(
                        out=kva[:, 2 * hh + mb, :], in_=ps[:, mb, :],
                        func=AF.Identity,
                        bias=csp_t[:, n0 + mb:n0 + mb + 1],
                    )
                    nc.vector.tensor_mul(
                        kvm[:, 2 * hh + mb, :], kva[:, 2 * hh + mb, :],
                        QT_t[:, n0 + mb, :],
                    )
                for ib in range(LB):
                    # fp8 PE transpose writes 16-bit granules: the output AP
                    # must step by 2 elements; the copy below compacts it.
                    psv = dv_ps.tile([128, 512], F8, tag="dv")
                    for db in range(2):
                        nc.tensor.transpose(
                            psv[:, db * 256:(db + 1) * 256:2],
                            VT_t[:, n0 + db, ib * 128:(ib + 1) * 128],
                            ident8,
                        )
                    nc.scalar.copy(
                        out=V_t[:, ib, h * DH:h * DH + 256],
                        in_=psv[:, 0:512:2],
                    )
            # pq = phi(q_mod): min -> Exp -> fused max+add STT
            qsl = slice(4 * g, 4 * g + 4)
            eq = hgrp.tile([128, 4, L], BF16, tag="eq")
            nc.vector.tensor_scalar_min(eq, kvm, 0.0)
            nc.scalar.activation(eq, eq, AF.Exp)
            nc.vector.scalar_tensor_tensor(
                out=pq_t[:, qsl, :], in0=kvm, scalar=0.0, in1=eq,
                op0=OP.max, op1=OP.add,
            )

        # ---- per-head: A matmul, mask, den, num, outT ----
        # attnT holds SA*attn in fp8 (|SA*attn| <= ~150 < 240 max)
        attnT_t = big.tile([128, NB, L], F8, tag="attnT")

        def head_block(h):
            n0 = 2 * h
            # causal block structure of AT[i, l] (i<=l kept):
            #   ib=0: l<128 lower-triangular, l>=128 all-ones
            #   ib=1: l<128 all-zero (skipped entirely), l>=128 triangular
            a_ps = an_ps.tile([128, 2, L], F32, tag="an")
            nc.tensor.matmul(
                a_ps[:, 0, :],
                pk_t[:, n0:n0 + 2, 0:128],
                pq_t[:, n0:n0 + 2, :],
                start=True, stop=True, perf_mode=DR,
            )
            nc.tensor.matmul(
                a_ps[:, 1, 128:L],
                pk_t[:, n0:n0 + 2, 128:L],
                pq_t[:, n0:n0 + 2, 128:L],
                start=True, stop=True, perf_mode=DR,
            )
            # am = A/SA in fp8; maskT rows 0:128 are 1/SA for l>=128 too,
            # so one multiply covers block 0's masked AND unmasked halves
            am = small.tile([128, LB, L], F8, tag="am")
            nc.vector.tensor_mul(am[:, 0, :], a_ps[:, 0, :], maskT_t[:, 0, :])
            nc.vector.tensor_mul(am[:, 1, 128:L], a_ps[:, 1, 128:L],
                                 maskT_t[:, 1, 128:L])

            den_ps = dv_ps.tile([128, L], F32, tag="dv", name="den_ps")
            nc.tensor.matmul(den_ps[:, 0:128], ones_t[:, 0, :], am[:, 0, 0:128],
                             start=True, stop=True)
            nc.tensor.matmul(den_ps[:, 128:L], ones_t, am[:, 0:2, 128:L],
                             start=True, stop=True, perf_mode=DR)
            # den is a sum of strictly positive phi-products (>= O(0.01)
            # mathematically, O(100) in practice), so the reference's 1e-8
            # clamp can never bind — reciprocal straight from PSUM.
            rden = small.tile([128, L], F32, tag="rden")
            nc.vector.reciprocal_approx_fast(out=rden, in_=den_ps)

            n_ps = an_ps.tile([128, 2, L], F32, tag="an")
            for db in range(2):
                dsl = slice(h * DH + db * 128, h * DH + (db + 1) * 128)
                nc.tensor.matmul(n_ps[:, db, 0:128], V_t[:, 0, dsl],
                                 am[:, 0, 0:128], start=True, stop=True)
                nc.tensor.matmul(n_ps[:, db, 128:L], V_t[:, 0:2, dsl],
                                 am[:, 0:2, 128:L],
                                 start=True, stop=True, perf_mode=DR)
            for db in range(2):
                nc.vector.tensor_mul(attnT_t[:, n0 + db, :], n_ps[:, db, :], rden)

        # ---- interleaved Q projection + attention pipeline ----
        # group g's whole chain only needs Q i-quarters 2g/2g+1, so its
        # DVE/ACT work hides under the next group's Q-proj matmuls.
        for g in range(4):
            proj_iq("q", 2 * g)
            proj_iq("q", 2 * g + 1)
            group_block(g)
            head_block(2 * g)
            head_block(2 * g + 1)

        # Trigger the sqrt ACT-table load now — after ScalarE's last
        # Copy/Exp user, off the LN tail's critical path (the set switch
        # costs ~2.6us).
        warm_sqrt = singles.tile([128, 1], F32)
        nc.scalar.activation(warm_sqrt, eps_t, AF.Sqrt)

        # ---- output projection + residual + LayerNorm ----
        # 512-wide moving operand: half the matmul and LDWEIGHTS count of
        # the input projections. All 8 panels are preloaded (wpool bufs=8)
        # so the loop can run lb-major: block 0's LayerNorm overlaps block
        # 1's matmuls instead of serializing at the tail.
        x_sb = [big.tile([128, D], F32, tag=f"x{lb}", name=f"x{lb}")
                for lb in range(LB)]
        stats = [small.tile([128, 4, 6], F32, tag=f"stats{lb}",
                            name=f"stats{lb}", bufs=1) for lb in range(LB)]
        wo_tiles = []
        for nq in range(4):
            wo = []
            for jh in range(2):
                w_t = wpool.tile([128, 8, 512], F8, tag="w",
                                 name=f"w_o{nq}{jh}")
                rows = slice((nq * 2 + jh) * 128, (nq * 2 + jh + 1) * 128)
                nc.sync.dma_start(
                    out=w_t.rearrange("p n i -> p (n i)"),
                    in_=w_ds["o"].ap()[rows, :])
                wo.append(w_t)
            wo_tiles.append(wo)

        def outproj_block(lb):
            for nq in range(4):
                wo = wo_tiles[nq]
                ps = psA.tile([128, 2, L], F32, tag="pk")
                psf = ps.rearrange("p a l -> p (a l)")
                for j in range(0, NB, 2):  # DoubleRow k-block pairs
                    nc.tensor.matmul(
                        psf,
                        attnT_t[:, j:j + 2, lb * 128:(lb + 1) * 128],
                        wo[j // 8][:, j % 8:j % 8 + 2, :],
                        start=(j == 0),
                        stop=(j == NB - 2),
                        perf_mode=DR,
                    )
                # x = o + (query + bo)
                sl = slice(nq * 512, (nq + 1) * 512)
                nc.vector.tensor_add(x_sb[lb][:, sl], psf, qres_t[lb][:, sl])
                # LN stats pipelined per 512-chunk while later chunks project
                nc.vector.bn_stats(out=stats[lb][:, nq, :],
                                   in_=x_sb[lb][:, sl])

        def ln_block(lb):
            x = x_sb[lb]
            x16 = big.tile([128, D], F16, tag=f"x16{lb}", name=f"x16{lb}")
            mv = small.tile([128, 2], F32, tag="mv")
            nc.vector.bn_aggr(out=mv, in_=stats[lb])
            sd = small.tile([128, 1], F32, tag="sd")
            nc.scalar.activation(sd, mv[:, 1:2], AF.Sqrt, bias=eps_t)
            nc.vector.reciprocal_approx_fast(out=sd, in_=sd)
            for ch in range(4):  # quarters, so DVE work overlaps output DMA
                sl = slice(ch * (D // 4), (ch + 1) * (D // 4))
                if plain_ln:
                    # ln_g == 1, ln_b == 0: fused (x - mu) * rstd. All on
                    # DVE: a ScalarE Identity here would force Sqrt<->
                    # Identity ACT-table swaps (~1.3us each) in the tail.
                    nc.vector.tensor_scalar(
                        out=x16[:, sl], in0=x[:, sl], scalar1=mv[:, 0:1],
                        scalar2=sd, op0=OP.subtract, op1=OP.mult,
                    )
                else:
                    nc.vector.tensor_scalar(
                        out=x[:, sl], in0=x[:, sl], scalar1=mv[:, 0:1],
                        scalar2=None, op0=OP.subtract,
                    )
                    nc.vector.scalar_tensor_tensor(
                        out=x[:, sl], in0=x[:, sl], scalar=sd, in1=lng_t[:, sl],
                        op0=OP.mult, op1=OP.mult,
                    )
                    nc.vector.tensor_add(x16[:, sl], x[:, sl], lnb_t[:, sl])
                # alternate output-DMA issue queues (both HWDGE: gpsimd's
                # SWDGE would starve behind the DVE LN ops here)
                oeng = nc.sync if ch % 2 == 0 else nc.scalar
                oeng.dma_start(
                    out=out_d.ap()[lb * 128:(lb + 1) * 128, sl], in_=x16[:, sl])

        for lb in range(LB):
            outproj_block(lb)
            ln_block(lb)


def _host_prep(query, key, value, Wq, Wk, Wv, Wo, bo, ln_g, ln_b, alpha, beta):
    """Host-side: cumulative_state shortcut + layout/dtype marshaling."""
    a, b = float(alpha), float(beta)
    f64 = np.float64
    # mean over (batch, l) of kv[b,h,l,m] = (1/(B*L)) sum_b Ksum[b,h,:].V[b,h,m,:]
    keysum = key.astype(f64).sum(axis=1)                      # [B, D]
    Ksum = (keysum @ Wk.T.astype(f64)).reshape(B, H, DH)      # [B, H, DH]
    WvH = Wv.astype(f64).reshape(H, DH, D)
    wv_eff = np.einsum("hdj,bhd->bhj", WvH, Ksum, optimize=True)      # [B,H,D]
    contrib = np.einsum("bmj,bhj->hm", value.astype(f64), wv_eff, optimize=True)
    mean_kv = contrib / (B * L)                               # [H, DH]
    cs = np.zeros((H, DH), f64)
    c = np.zeros(DH, f64)
    for h in range(H):
        cs[h] = c
        c = b * c + a * mean_kv[h]
    # q_mod = Q*((1-a)*cs + a*kv) = (a*Q)*(kv + (1-a)/a*cs); a is folded
    # into the Q PSUM-copy scale on device, and this is cs*(1-a)/a:
    csp = ((1.0 - a) / a * cs if a != 0 else 0.0 * cs).astype(np.float32)
    csp_dev = np.ascontiguousarray(
        csp.reshape(H, 2, 128).transpose(2, 0, 1).reshape(128, H * 2)
    )
    plain_ln = bool(np.all(ln_g == 1.0) and np.all(ln_b == 0.0))

    bf = ml_dtypes.bfloat16
    f8 = ml_dtypes.float8_e4m3  # TRN fp8e4: max 240, matches bit-for-bit

    def to8(x):
        return np.clip(x, -240.0, 240.0).astype(f8)

    # pack into the exact SBUF tile layouts (one contiguous run per
    # partition per DMA): proj panels [iq*128+p, n*256+c] = wT[n*128+p,
    # iq*256+c]; outproj [(nq*2+jh)*128+p, k*512+c] = woT[(jh*8+k)*128+p,
    # nq*512+c]; activations [p, n*L+l] = xT[n*128+p, l].
    def pack_w(wT):
        arr = np.asarray(wT).reshape(NB, 128, 8, 256)
        return np.ascontiguousarray(
            arr.transpose(2, 1, 0, 3).reshape(8 * 128, NB * 256))

    def pack_wo(woT_):
        arr = np.asarray(woT_).reshape(2, 8, 128, 4, 512)
        return np.ascontiguousarray(
            arr.transpose(3, 0, 2, 1, 4).reshape(8 * 128, 8 * 512))

    def pack_x(xT):
        arr = np.asarray(xT).reshape(NB, 128, L)
        return np.ascontiguousarray(arr.transpose(1, 0, 2).reshape(128, NB * L))

    qT = np.stack([pack_x(to8(query[c].T)) for c in range(B)])
    kT = np.stack([pack_x(to8(key[c].T)) for c in range(B)])
    vT = np.stack([pack_x(to8(value[c].T)) for c in range(B)])
    wqT = pack_w(to8(SW * Wq.T))
    wkT = pack_w(to8(SW * Wk.T))
    wvT = pack_w(to8(SW * Wv.T))
    woT = pack_wo(to8(SW * Wo.T))
    # out-proj PSUM is SW*SA*o; scaling the residual to match makes
    # x_dev = SW*SA*x, and LayerNorm is invariant to uniform scaling.
    # fp16 holds it fine: |SW*SA*x| <= ~2048*6 << 65504.
    qres = (SW * SA * (query + bo[None, None, :])).astype(np.float16)
    # mask[i,l] = 1/SA iff i<=l: folds the fp8 am = A/SA scale into the mask
    mask_full = np.triu(np.full((L, L), 1.0 / SA, np.float32))
    maskT = np.ascontiguousarray(
        mask_full.reshape(LB, 128, L).transpose(1, 0, 2).reshape(128, LB * L)
    ).astype(bf)

    in_maps = []
    for c_ in range(B):
        in_maps.append({
            "qT": qT[c_], "kT": kT[c_], "vT": vT[c_],
            "qres": qres[c_],
            "wqT": wqT, "wkT": wkT, "wvT": wvT, "woT": woT,
            "csp": csp_dev, "maskT": maskT,
            "lng": ln_g.astype(np.float32), "lnb": ln_b.astype(np.float32),
        })
    return in_maps, a, plain_ln


def get_nc(alpha: float, plain_ln: bool = True):
    key = (round(float(alpha), 9), bool(plain_ln))
    if key not in _cache:
        _cache[key] = _build(float(alpha), bool(plain_ln))
    return _cache[key]


def kernel(query, key, value, Wq, Wk, Wv, Wo, bo, ln_g, ln_b, alpha, beta,
           _trace=False, _trace_kwargs=None):
    args = [np.asarray(x) for x in
            (query, key, value, Wq, Wk, Wv, Wo, bo, ln_g, ln_b, alpha, beta)]
    in_maps, a, plain_ln = _host_prep(*args)
    nc = get_nc(a, plain_ln)
    res = run_bass_kernel_spmd(
        nc, in_maps, core_ids=list(range(B)),
        trace=_trace, **(_trace_kwargs or {}),
    )
    out = np.stack([res.results[c]["out"] for c in range(B)], axis=0)
    out = out.astype(np.float32)
    if _trace:
        kernel._last_results = res
    return out

